# revision 1
# baseline (speedup 1.0000x reference)
"""TRN2 Bass kernel for nn_CausalSelfAttention_4054449128214.

The reference returns out_s + stop_gradient(out_full - out_s), whose forward
value is exactly out_full — plain dense causal self-attention. So the kernel
computes: qkv = x@W_attn+b_attn, per-head causal softmax attention, y@W_proj+b_proj.

Sharding (8 cores, no collectives):
  Megatron head-parallel. Cores 0-3 own head pairs (0,1)..(6,7); cores 4-7 own
  heads 8..11 (run twice for SPMD shape-uniformity, second copy's W_proj rows
  zeroed). Each core computes its heads' QKV columns, attention, and a partial
  row-sliced output projection; the host sums the 8 partials (the Megatron
  row-parallel all-reduce) and transposes back.

All matmuls run as float32r (TF32-class, ~13-bit mantissa, full PE rate at
free-dim >= 256); accumulation is exact fp32 in PSUM.
"""

import numpy as np

import concourse.bacc as bacc
import concourse.mybir as mybir
import concourse.tile as tile
from concourse.bass_utils import run_bass_kernel_spmd

F32 = mybir.dt.float32
F32R = mybir.dt.float32r

T = 1024          # sequence length
C = 768           # channels
NH = 12           # heads
HS = 64           # head size
NCORES = 8
TT = 512          # t-tile (matmul moving free dim)
NT = T // TT      # 2
NCC = C // 128    # 6 contraction chunks
NKC = T // 128    # 8 key chunks
SCALE = 1.0 / 8.0  # 1/sqrt(HS)

# core -> (head0, head1); cores 4-7 duplicate their head (2nd W_proj slice zeroed)
HEAD_MAP = [(0, 1), (2, 3), (4, 5), (6, 7), (8, 8), (9, 9), (10, 10), (11, 11)]

_CACHE: dict = {}


def _build_program():
    nc = bacc.Bacc("TRN2", target_bir_lowering=False, debug=False,
                   num_devices=NCORES)
    xT = nc.dram_tensor("xT", [C, T], F32, kind="ExternalInput").ap()
    wsel = nc.dram_tensor("wsel", [C, 384], F32, kind="ExternalInput").ap()
    wp = nc.dram_tensor("wp", [128, C], F32, kind="ExternalInput").ap()
    bqk = nc.dram_tensor("bqk", [128, 3], F32, kind="ExternalInput").ap()
    bpr = nc.dram_tensor("bpr", [128, NCC], F32, kind="ExternalInput").ap()
    eye2 = nc.dram_tensor("eye2", [128, HS], F32, kind="ExternalInput").ap()
    ones = nc.dram_tensor("ones", [128, 130], F32, kind="ExternalInput").ap()
    outT = nc.dram_tensor("outT", [C, T], F32, kind="ExternalOutput").ap()

    with tile.TileContext(nc) as tc:
        with (
            tc.tile_pool(name="const", bufs=1) as cp,
            tc.tile_pool(name="e", bufs=8) as ep,
            tc.tile_pool(name="rb", bufs=4) as rbp,
            tc.tile_pool(name="pmm", bufs=2, space="PSUM") as pmm,
            tc.tile_pool(name="pst", bufs=3, space="PSUM") as pst,
            tc.tile_pool(name="pov", bufs=2, space="PSUM") as pov,
            tc.tile_pool(name="ptr", bufs=1, space="PSUM") as ptr,
        ):
            # ---- big merged loads on SP; small constants on Pool's queue ----
            wt = cp.tile([128, NCC * 384], F32R, tag="wt")
            xt0 = cp.tile([128, NCC * TT], F32R, tag="xt0")
            xt1 = cp.tile([128, NCC * TT], F32R, tag="xt1")
            wsel3 = wsel.rearrange("(c p) j -> p c j", p=128).bitcast(F32R)
            x03 = xT[:, 0:TT].rearrange("(c p) t -> p c t", p=128).bitcast(F32R)
            x13 = xT[:, TT:T].rearrange("(c p) t -> p c t", p=128).bitcast(F32R)
            # interleave thirds so chunk cc lands early
            for c0, c1 in ((0, 2), (2, 4), (4, 6)):
                nc.sync.dma_start(
                    out=wt[:].rearrange("p (c j) -> p c j", c=NCC)[:, c0:c1],
                    in_=wsel3[:, c0:c1])
                nc.sync.dma_start(
                    out=xt0[:].rearrange("p (c t) -> p c t", c=NCC)[:, c0:c1],
                    in_=x03[:, c0:c1])
            for pc in range(3):
                c0, c1 = pc * 2, pc * 2 + 2
                nc.sync.dma_start(
                    out=xt1[:].rearrange("p (c t) -> p c t", c=NCC)[:, c0:c1],
                    in_=x13[:, c0:c1])
            wpt = cp.tile([64, 2 * C], F32R, tag="wpt")
            nc.sync.dma_start(
                out=wpt[:].rearrange("p (h e) -> p h e", h=2),
                in_=wp.rearrange("(h p) e -> p h e", p=64).bitcast(F32R))
            xts = [[xt0[:, cc * TT:(cc + 1) * TT], xt1[:, cc * TT:(cc + 1) * TT]]
                   for cc in range(NCC)]
            ws = [wt[:, cc * 384:(cc + 1) * 384] for cc in range(NCC)]
            wps = [wpt[:, hi * C:(hi + 1) * C] for hi in range(2)]

            bqk_sb = cp.tile([128, 3], F32, tag="bqk")
            nc.gpsimd.dma_start(out=bqk_sb[:], in_=bqk)
            eye_sb = cp.tile([128, HS], F32R, tag="eye")
            nc.gpsimd.dma_start(out=eye_sb[:], in_=eye2.bitcast(F32R))
            ones_sb = cp.tile([128, 2], F32R, tag="ones_sb")
            nc.gpsimd.dma_start(out=ones_sb[:], in_=ones[:, 0:2].bitcast(F32R))
            vaug = [cp.tile([128, 130], F32R, tag=f"va{kc}", name=f"va{kc}")
                    for kc in range(NKC)]
            for kc in range(NKC):
                # ones columns at 64 and 129 (cols 0:64 / 65:129 overwritten later)
                nc.vector.tensor_copy(
                    vaug[kc][:, 64:130:65], ones_sb[:])
            bpr_sb = cp.tile([128, NCC], F32, tag="bpr")
            nc.gpsimd.dma_start(out=bpr_sb[:], in_=bpr)

            # static causal masks for the DVE half of the mask work
            masks = []
            for kcr in range(4):
                m = cp.tile([128, TT], F32, tag=f"mask{kcr}", name=f"mask{kcr}")
                nc.vector.memset(m[:], 1.0)
                nc.gpsimd.affine_select(
                    m[:], m[:], pattern=[[1, TT]],
                    compare_op=mybir.AluOpType.is_ge, fill=0.0,
                    base=-128 * kcr, channel_multiplier=-1)
                masks.append(m)

            qkvT = [[None] * NT for _ in range(3)]
            yT = [[None] * NT for _ in range(2)]
            ost = [cp.tile([128, 2 * TT], F32, tag=f"ost{tt}{h}", name=f"ost{tt}{h}")
                   for tt in range(NT) for h in range(3)]

            def emit_qkv(tt):
                for mt in (2, 0, 1):
                    qkvT[mt][tt] = cp.tile([128, TT], F32R, tag=f"qkv{mt}_{tt}",
                                           name=f"qkv{mt}_{tt}")
                    ps = pmm.tile([128, TT], F32, tag="mm")
                    for cc in range(NCC):
                        nc.tensor.matmul(
                            ps[:], ws[cc][:, mt * 128:(mt + 1) * 128],
                            xts[cc][tt], start=(cc == 0), stop=(cc == NCC - 1))
                    nc.vector.tensor_scalar_add(
                        qkvT[mt][tt][:], ps[:], bqk_sb[:, mt:mt + 1])

            def emit_vaug(tt):
                for kc in range(tt * 4, tt * 4 + 4):
                    col = (kc % 4) * 128
                    for hi in range(2):
                        pt = ptr.tile([128, HS], F32R, tag="pt")
                        nc.tensor.transpose(
                            pt[:], qkvT[2][tt][hi * 64:(hi + 1) * 64, col:col + 128],
                            eye_sb[hi * 64:(hi + 1) * 64, :])
                        nc.vector.tensor_copy(vaug[kc][:, hi * 65:hi * 65 + 64], pt[:])

            def emit_attn(qt):
                for hi in range(2):
                    nlive = qt * 4 + 4
                    po = pov.tile([65, TT], F32, tag="po")
                    for kc in range(nlive):
                        ktile = qkvT[1][kc // 4]
                        kcol = (kc % 4) * 128
                        ps = pst.tile([128, TT], F32, tag="st")
                        nc.tensor.matmul(
                            ps[:], ktile[hi * 64:(hi + 1) * 64, kcol:kcol + 128],
                            qkvT[0][qt][hi * 64:(hi + 1) * 64, :],
                            start=True, stop=True)
                        e = ep.tile([128, TT], F32R, tag="e")
                        nc.scalar.activation(
                            e[:], ps[:], mybir.ActivationFunctionType.Exp,
                            scale=SCALE)
                        kcr = kc - qt * 4
                        if kcr >= 0:  # diagonal chunk: zero where tk > tq
                            if kcr % 2 == 0:
                                nc.gpsimd.affine_select(
                                    e[:], e[:], pattern=[[1, TT]],
                                    compare_op=mybir.AluOpType.is_ge, fill=0.0,
                                    base=-128 * kcr, channel_multiplier=-1)
                            else:
                                nc.vector.tensor_mul(e[:], e[:], masks[kcr][:])
                        nc.tensor.matmul(
                            po[:], vaug[kc][:, hi * 65:(hi + 1) * 65], e[:],
                            start=(kc == 0), stop=(kc == nlive - 1))
                    rb = rbp.tile([128, TT], F32, tag="rb")
                    nc.vector.reciprocal(rb[0:1, :], po[64:65, :])
                    rbc = rbp.tile([64, TT], F32, tag="rbc")
                    nc.gpsimd.partition_broadcast(rbc[:], rb[0:1, :])
                    yT[hi][qt] = cp.tile([64, TT], F32R, tag=f"y{hi}_{qt}",
                                         name=f"y{hi}_{qt}")
                    nc.vector.tensor_mul(yT[hi][qt][:], po[0:64, :], rbc[:])

            def emit_proj(tt):
                for half in range(3):
                    stile = ost[tt * 3 + half]
                    for ei in range(2):
                        et = half * 2 + ei
                        pm = pmm.tile([128, TT], F32, tag="mm")
                        for hi in range(2):
                            nc.tensor.matmul(
                                pm[:], wps[hi][:, et * 128:(et + 1) * 128],
                                yT[hi][tt][:], start=(hi == 0), stop=(hi == 1))
                        dst = stile[:, ei * TT:(ei + 1) * TT]
                        if et % 2 == 0:
                            nc.scalar.activation(
                                dst, pm[:], mybir.ActivationFunctionType.Identity,
                                bias=bpr_sb[:, et:et + 1])
                        else:
                            nc.vector.tensor_scalar_add(dst, pm[:], bpr_sb[:, et:et + 1])
                    nc.sync.dma_start(
                        out=outT[half * 256:(half + 1) * 256, tt * TT:(tt + 1) * TT]
                        .rearrange("(g p) t -> p g t", p=128),
                        in_=stile[:].rearrange("p (g t) -> p g t", g=2))

            emit_qkv(0)
            emit_vaug(0)
            emit_attn(0)
            emit_qkv(1)
            emit_vaug(1)
            emit_proj(0)
            emit_attn(1)
            emit_proj(1)
    nc.compile()
    return nc


def _in_maps(x, W_attn, b_attn, W_proj, b_proj):
    xTn = np.ascontiguousarray(x.reshape(T, C).T)  # [C, T]
    eye2 = np.ascontiguousarray(np.tile(np.eye(HS, dtype=np.float32), (2, 1)))
    maps = []
    for core in range(NCORES):
        h0, h1 = HEAD_MAP[core]
        cols = []
        for part in range(3):  # q, k, v column groups of W_attn
            for h in (h0, h1):
                cols.extend(range(part * C + h * HS, part * C + (h + 1) * HS))
        wsel = np.ascontiguousarray(W_attn[:, cols])                    # [C, 384]
        bqk = np.stack(
            [np.concatenate([b_attn[p * C + h0 * HS:p * C + (h0 + 1) * HS],
                             b_attn[p * C + h1 * HS:p * C + (h1 + 1) * HS]])
             for p in range(3)], axis=1).astype(np.float32)             # [128, 3]
        wpc = np.concatenate(
            [W_proj[h0 * HS:(h0 + 1) * HS, :],
             np.zeros_like(W_proj[:HS]) if h1 == h0
             else W_proj[h1 * HS:(h1 + 1) * HS, :]], axis=0)            # [128, C]
        bpr = (b_proj.reshape(NCC, 128).T if core == 0
               else np.zeros((128, NCC), np.float32)).astype(np.float32)
        maps.append({
            "xT": xTn, "wsel": np.ascontiguousarray(wsel.astype(np.float32)),
            "wp": np.ascontiguousarray(wpc.astype(np.float32)),
            "bqk": np.ascontiguousarray(bqk), "bpr": np.ascontiguousarray(bpr),
            "eye2": eye2, "ones": np.ones((128, 130), np.float32),
        })
    return maps


def kernel(x, W_attn, b_attn, W_proj, b_proj, _trace=False, _trace_kwargs=None):
    x = np.asarray(x, np.float32)
    W_attn = np.asarray(W_attn, np.float32)
    b_attn = np.asarray(b_attn, np.float32)
    W_proj = np.asarray(W_proj, np.float32)
    b_proj = np.asarray(b_proj, np.float32)

    if "nc" not in _CACHE:
        _CACHE["nc"] = _build_program()
    nc = _CACHE["nc"]

    maps = _in_maps(x, W_attn, b_attn, W_proj, b_proj)
    kw = {}
    if _trace:
        kw = dict(trace=True, **(_trace_kwargs or {}))
    br = run_bass_kernel_spmd(nc, maps, list(range(NCORES)), **kw)
    acc = np.zeros((C, T), np.float64)
    for core in range(NCORES):
        acc += br.results[core]["outT"].astype(np.float64)
    out = np.ascontiguousarray(acc.T.astype(np.float32)).reshape(1, T, C)
    _CACHE["last_results"] = br
    return out



# revision 53
# speedup vs baseline: 1.3783x; 1.3783x over previous
"""TRN2 Bass kernel for nn_CausalSelfAttention_4054449128214.

The reference returns out_s + stop_gradient(out_full - out_s), whose forward
value is exactly out_full -- plain dense causal self-attention. So the kernel
computes: qkv = x@W_attn+b_attn, per-head causal softmax attention,
y@W_proj+b_proj.

Sharding (8 cores, no collectives):
  Megatron head-parallel. Cores 0-3 own head pairs (0,1)..(6,7); cores 4-7 own
  heads 8..11 (duplicated for SPMD shape-uniformity, second copy's W_proj rows
  zeroed). Each core computes its heads' QKV, attention, and a row-sliced
  partial output projection; the host sums the 8 partials (the Megatron
  row-parallel all-reduce).

v2 dataflow (all bf16 operands, fp32 PSUM accumulation):
  - Inputs host-converted to bf16, partition-major: every DMA is an identity
    copy with >=512B contiguous runs. x chunks stream in; QKV runs cc-major
    across 4 PSUM chains so compute starts with the first chunk.
  - K^T Q scores are causal-tight (valid query suffix per 128-key chunk),
    exp on the scalar engine writes bf16 e-tiles, diagonal masked by one
    bf16 tri multiply on DVE.
  - V^T comes straight from swapped-operand matmuls (no transposes).
  - PV uses the transposed orientation y^T[q,65] = e^T @ [V|1]: 65-row
    matmuls; softmax normalization is a per-partition reciprocal +
    tensor_scalar (no partition broadcast).
  - y^T transposes back per 128-query block on the PE; the projection
    contracts both heads in one 128-deep matmul, in four 256-query passes
    so stores overlap the attention tail.
  - q/k biases ride the mandatory PSUM->SBUF copies; the v bias and b_proj
    are folded on the host exactly (softmax weights sum to 1).
"""

import numpy as np
import ml_dtypes

import concourse.bacc as bacc
import concourse.mybir as mybir
import concourse.tile as tile
from concourse.bass_utils import run_bass_kernel_spmd

F32 = mybir.dt.float32
BF16 = mybir.dt.bfloat16
EXP = mybir.ActivationFunctionType.Exp
IDENT = mybir.ActivationFunctionType.Identity
COPY = mybir.ActivationFunctionType.Copy

T = 1024          # sequence length
C = 768           # channels
NH = 12           # heads
HS = 64           # head size
NCORES = 8
NCC = C // 128    # 6 contraction chunks
NKC = T // 128    # 8 key chunks of 128
SCALE = 1.0 / 8.0  # 1/sqrt(HS)
BF = np.dtype(ml_dtypes.bfloat16)

# core -> (head0, head1); cores 4-7 duplicate their head (2nd W_proj slice zeroed)
HEAD_MAP = [(0, 1), (2, 3), (4, 5), (6, 7), (8, 8), (9, 9), (10, 10), (11, 11)]

_CACHE: dict = {}


def _build_program():
    nc = bacc.Bacc("TRN2", target_bir_lowering=False, debug=False,
                   num_devices=NCORES)
    xd = nc.dram_tensor("xd", [128, NCC * T], BF16, kind="ExternalInput").ap()
    # packed row: bqk f32[2] | wqk bf16[1536]
    wqk = nc.dram_tensor("wqk", [128, 4 + NCC * 256], BF16,
                         kind="ExternalInput").ap()
    # packed row: eye bf16[128] | tri bf16[128] | wv bf16[768]
    cst = nc.dram_tensor("cst", [128, 1024], BF16, kind="ExternalInput").ap()
    wp = nc.dram_tensor("wp", [128, C], BF16, kind="ExternalInput").ap()
    outp = nc.dram_tensor("outp", [128, NCC * T], BF16, kind="ExternalOutput").ap()

    with tile.TileContext(nc) as tc:
        with (
            tc.tile_pool(name="const", bufs=1) as cp,
            tc.tile_pool(name="pm", bufs=2, space="PSUM") as pm,
            tc.tile_pool(name="psc", bufs=2, space="PSUM") as psc,
            tc.tile_pool(name="pvt", bufs=1, space="PSUM") as pvt,
            tc.tile_pool(name="pyp", bufs=1, space="PSUM") as pyp,
            tc.tile_pool(name="ptr", bufs=1, space="PSUM") as ptr,
        ):
            # vaug: per key-chunk [V_h0|1|V_h1|1] (65 cols per head), bf16
            vaug = cp.tile([128, NKC * 130], BF16, tag="vaug")
            va3 = vaug[:].rearrange("p (k j) -> p k j", k=2 * NKC)
            nc.gpsimd.memset(va3[:, :, 64:65], 1.0)

            # warmup: load the Exp table on Act and spin the PE p-state up
            # during the DMA window (dummy transposes on a memset scratch)
            wsc = cp.tile([128, 128], BF16, tag="wsc")
            nc.gpsimd.memset(wsc[:, 0:128], 0.0)
            wscf = cp.tile([128, 1], F32, tag="wscf")
            nc.vector.memset(wscf[:], 0.0)
            wsce = cp.tile([128, 1], BF16, tag="wsce")
            nc.scalar.activation(wsce[:], wscf[:], EXP, scale=1.0)

            # ---- big loads on the SP queue (identity copies) ----
            # DMA cadence floor is ~650ns (HWDGE setup), so pieces are >=2
            # chunks; tt0 halves of x stream first so the tt0 QKV chains,
            # scores, and exp start ~3us earlier.
            wqk_sb = cp.tile([128, 4 + NCC * 256], BF16, tag="wqk")
            bqk_sb = wqk_sb[:, 0:4].bitcast(F32)         # [128, 2] f32
            wqk3s = wqk_sb[:, 4:].rearrange("p (c j) -> p c j", c=NCC)
            x_sb = cp.tile([128, NCC * T], BF16, tag="x")
            x3 = x_sb[:].rearrange("p (c t) -> p c t", c=NCC)
            cst_sb = cp.tile([128, 1024], BF16, tag="cst")
            wp_sb = cp.tile([128, C], BF16, tag="wp")

            eye_sb = cst_sb[:, 0:128]                    # [128, 128] bf16
            ltri_sb = cst_sb[:, 128:256]                 # -1e5 upper tri
            wv3 = cst_sb[:, 256:1024].rearrange("p (c j) -> p c j", c=NCC)

            x4 = x_sb[:].rearrange("p (c t u) -> p c t u", c=NCC, t=2)
            xd4 = xd.rearrange("p (c t u) -> p c t u", c=NCC, t=2)
            nc.sync.dma_start(out=wqk_sb[:], in_=wqk)
            nc.sync.dma_start(out=x4[:, 0:2, 0], in_=xd4[:, 0:2, 0])
            nc.sync.dma_start(out=x4[:, 2:4, 0], in_=xd4[:, 2:4, 0])
            nc.sync.dma_start(out=x4[:, 4:6, 0], in_=xd4[:, 4:6, 0])
            nc.sync.dma_start(out=cst_sb[:], in_=cst)
            for c0 in (0, 2, 4):
                nc.sync.dma_start(out=x4[:, c0:c0 + 2, 1], in_=xd4[:, c0:c0 + 2, 1])
            nc.sync.dma_start(out=wp_sb[:], in_=wp)

            q_sb = cp.tile([128, T], BF16, tag="q_sb")
            k_sb = cp.tile([128, T], BF16, tag="k_sb")
            y_sb = cp.tile([128, T], BF16, tag="y_sb")
            # e tiles: per (head, kc), width = T - 128*kc
            e_t = [[cp.tile([128, T - 128 * kc], BF16, tag=f"e{h}_{kc}",
                            name=f"e{h}_{kc}") for kc in range(NKC)]
                   for h in range(2)]
            ytn = cp.tile([128, 256], BF16, tag="ytn")   # 4 regions of 64
            rec = cp.tile([128, 4], F32, tag="rec")      # 2 regions of 2
            ost = [cp.tile([128, NCC * 512], BF16, tag=f"ost{tt}", name=f"ost{tt}")
                   for tt in range(2)]
            # persistent PSUM tiles. HW zeroes a whole 2KB bank on matmul
            # start, so each bank holds exactly ONE live accumulation region,
            # reused in place (same-address WAR deps serialize reuse).
            vt_ps = pvt.tile([128, 128], F32, tag="vt")
            yp0_ps = pyp.tile([128, 65], F32, tag="yp0", name="yp0")
            yp1_ps = pyp.tile([128, 65], F32, tag="yp1", name="yp1")
            tr_ps = ptr.tile([128, 128], BF16, tag="tr")

            # PE p-state warmup: ~36 dummy transposes fill the DMA window so
            # QKV starts at full clock (ramp needs 3us of continuous busy)
            for _ in range(36):
                nc.tensor.transpose(tr_ps[:], wsc[:], wsc[:])

            # ---- QKV: cc-major chain pairs; pm bufs rotate tt0 -> tt1 ----
            def emit_qk(tt):
                ps = [pm.tile([128, 512], F32, tag="mm", name=f"q{tt}ps"),
                      pm.tile([128, 512], F32, tag="mm", name=f"k{tt}ps")]
                for cc in range(NCC):
                    for mt in range(2):
                        nc.tensor.matmul(
                            ps[mt][:], wqk3s[:, cc, mt * 128:(mt + 1) * 128],
                            x3[:, cc, tt * 512:(tt + 1) * 512],
                            start=(cc == 0), stop=(cc == NCC - 1))
                qdst = q_sb[:, tt * 512:(tt + 1) * 512]
                kdst = k_sb[:, tt * 512:(tt + 1) * 512]
                if tt == 0:
                    nc.scalar.activation(qdst, ps[0][:], IDENT,
                                         bias=bqk_sb[:, 0:1])
                    nc.vector.tensor_scalar_add(kdst[:, 0:128], ps[1][:, 0:128],
                                                bqk_sb[:, 1:2])
                    nc.vector.tensor_scalar_add(kdst[:, 128:512],
                                                ps[1][:, 128:512],
                                                bqk_sb[:, 1:2])
                else:
                    nc.vector.tensor_scalar_add(qdst, ps[0][:], bqk_sb[:, 0:1])
                    nc.vector.tensor_scalar_add(kdst, ps[1][:], bqk_sb[:, 1:2])

            def emit_vt(kc):
                # vT[key, (h0 hs | h1 hs)] for this 128-key chunk, then
                # scatter into vaug (the copy also frees the bank for kc+1)
                for cc in range(NCC):
                    nc.tensor.matmul(
                        vt_ps[:], x3[:, cc, kc * 128:(kc + 1) * 128], wv3[:, cc],
                        start=(cc == 0), stop=(cc == NCC - 1))
                dst = va3[:, 2 * kc:2 * kc + 2, 0:64]
                nc.vector.tensor_copy(dst,
                                      vt_ps[:].rearrange("p (h j) -> p h j", h=2))

            def emit_scores(h, kc, piece):
                # piece 0: q in [128kc, 512); piece 1: q in [512, 1024)
                # kc >= 4 only has piece 1 (q in [128kc, 1024))
                if kc < 4:
                    qlo, qhi = (128 * kc, 512) if piece == 0 else (512, T)
                else:
                    qlo, qhi = 128 * kc, T
                w = qhi - qlo
                diag = piece == 0 or kc >= 4
                ps = psc.tile([128, 512], F32, tag="sc")
                nc.tensor.matmul(
                    ps[:, 0:w], k_sb[h * 64:h * 64 + 64, kc * 128:kc * 128 + 128],
                    q_sb[h * 64:h * 64 + 64, qlo:qhi], start=True, stop=not diag)
                if diag:
                    # causal mask on the PE: accumulate -1e5 into the upper
                    # triangle of the diagonal block (ltri^T @ I)
                    nc.tensor.matmul(ps[:, 0:128], ltri_sb, eye_sb,
                                     start=False, stop=True)
                off = qlo - 128 * kc
                nc.scalar.activation(e_t[h][kc][:, off:off + w], ps[:, 0:w],
                                     EXP, scale=SCALE)

            y7 = [None, None]

            def yreg(h, qc):
                if qc == 7:
                    return y7[h][:, 0:65]
                if qc == 6:
                    return vt_ps[:, 0:65]
                return (yp0_ps if h == 0 else yp1_ps)[:]

            def emit_pv_mms(qc, kcs, last, heads=(0, 1)):
                for h in heads:
                    reg = yreg(h, qc)
                    for kc in kcs:
                        nc.tensor.matmul(
                            reg,
                            e_t[h][kc][:, 128 * (qc - kc):128 * (qc - kc) + 128],
                            vaug[:, 130 * kc + 65 * h:130 * kc + 65 * h + 65],
                            start=(kc == 0), stop=(last and kc == kcs[-1]))

            def emit_pv_norm1(qc, h):
                # reciprocal of the denominator column, then scale; late qcs
                # put h1 on Act (free after the exps) to shorten the DVE tail
                rr = (qc % 2) * 2
                nc.vector.reciprocal(rec[:, rr + h:rr + h + 1],
                                     yreg(h, qc)[:, 64:65])
                ydst = ytn[:, ((qc % 2) * 2 + h) * 64:((qc % 2) * 2 + h) * 64 + 64]
                if h == 1 and qc >= 6:
                    nc.scalar.activation(ydst, yreg(h, qc)[:, 0:64], COPY,
                                         scale=rec[:, rr + h:rr + h + 1])
                else:
                    nc.vector.tensor_scalar_mul(ydst, yreg(h, qc)[:, 0:64],
                                                rec[:, rr + h:rr + h + 1])

            def emit_pv_norm(qc):
                for h in range(2):
                    emit_pv_norm1(qc, h)

            def emit_pv(qc):
                # y^T[q, 0:64]=numerator, [:,64]=denominator, 128 queries/head
                if qc == 6:
                    # both heads share the vt bank: norm h0 before h1's chain
                    emit_pv_mms(qc, list(range(qc + 1)), True, heads=(0,))
                    emit_pv_norm1(qc, 0)
                    emit_pv_mms(qc, list(range(qc + 1)), True, heads=(1,))
                    emit_pv_norm1(qc, 1)
                else:
                    emit_pv_mms(qc, list(range(qc + 1)), True)
                    emit_pv_norm(qc)

            def emit_post(qc):
                # one transpose moves both heads' [q, hs] block to [hs2, q]
                nc.tensor.transpose(
                    tr_ps[:], ytn[:, (qc % 2) * 128:(qc % 2) * 128 + 128],
                    eye_sb)
                dst = y_sb[:, qc * 128:qc * 128 + 128]
                if qc >= 6:
                    nc.scalar.activation(dst, tr_ps[:], COPY)
                else:
                    nc.vector.tensor_copy(dst, tr_ps[:])

            def emit_proj(qp, engs=(nc.vector, nc.vector, nc.vector),
                          split_store=False, pool=None):
                # one 256-query pass over all 6 row-chunks; 2 chunks share a
                # PSUM bank so each copy moves [128,512]
                for eh in range(3):
                    p = pool[eh] if isinstance(pool, list) else (pool or pm)
                    ps = p.tile([128, 512], F32,
                                tag="mm" if p is pm else "sc")
                    for ei in range(2):
                        et = eh * 2 + ei
                        nc.tensor.matmul(
                            ps[:, ei * 256:ei * 256 + 256],
                            wp_sb[:, et * 128:(et + 1) * 128],
                            y_sb[:, qp * 256:(qp + 1) * 256],
                            start=True, stop=True)
                    dst = ost[qp // 2][:].rearrange(
                        "p (e t) -> p e t", e=NCC)[:, eh * 2:eh * 2 + 2,
                                                   (qp % 2) * 256:(qp % 2) * 256 + 256]
                    eng = engs[eh]
                    if eng is nc.scalar:
                        nc.scalar.activation(
                            dst, ps[:].rearrange("p (e t) -> p e t", e=2), COPY)
                    else:
                        eng.tensor_copy(dst,
                                        ps[:].rearrange("p (e t) -> p e t", e=2))
                outd = outp.rearrange("p (e t) -> p e t", e=NCC)
                osts = ost[qp // 2][:].rearrange("p (e t) -> p e t", e=NCC)
                qs = slice(qp * 256, (qp + 1) * 256)
                ls = slice((qp % 2) * 256, (qp % 2) * 256 + 256)
                if split_store:
                    # one store per copy so the tail DMA starts ASAP
                    for eh in range(3):
                        nc.sync.dma_start(out=outd[:, 2 * eh:2 * eh + 2, qs],
                                          in_=osts[:, 2 * eh:2 * eh + 2, ls])
                else:
                    nc.sync.dma_start(out=outd[:, :, qs], in_=osts[:, :, ls])

            # ---------------- schedule ----------------
            emit_qk(0)
            for kc in range(4):
                for h in range(2):
                    emit_scores(h, kc, 0)
            emit_vt(0)
            emit_vt(1)
            emit_vt(2)
            emit_vt(3)
            emit_qk(1)
            emit_pv(0)
            emit_pv(1)
            emit_post(0)
            for h in range(2):
                emit_scores(h, 0, 1)
            for h in range(2):
                emit_scores(h, 1, 1)
            emit_pv(2)
            emit_post(1)
            emit_proj(0)
            for h in range(2):
                emit_scores(h, 2, 1)
            for h in range(2):
                emit_scores(h, 3, 1)
            emit_pv(3)
            emit_post(2)
            for h in range(2):
                emit_scores(h, 4, 1)
            emit_vt(4)
            for h in range(2):
                emit_scores(h, 5, 1)
            emit_vt(5)
            for h in range(2):
                emit_scores(h, 6, 1)
            for h in range(2):
                emit_scores(h, 7, 1)
            emit_vt(6)
            emit_vt(7)
            emit_pv(4)
            emit_post(3)
            emit_proj(1)
            emit_pv(5)
            emit_post(4)
            emit_pv(6)
            emit_post(5)
            # chain 7 accumulates in freshly-freed psc banks so it does
            # not wait for the yp-bank norm ladder
            y7[0] = psc.tile([128, 512], F32, tag="sc", name="y7a")
            y7[1] = psc.tile([128, 512], F32, tag="sc", name="y7b")
            emit_pv(7)
            emit_post(6)
            emit_post(7)
            emit_proj(2, engs=(nc.scalar, nc.vector, nc.vector), pool=psc)
            emit_proj(3, engs=(nc.scalar, nc.vector, nc.scalar),
                      pool=[pm, pm, psc])
    nc.compile()
    return nc


def _in_maps(x, W_attn, b_attn, W_proj, b_proj):
    x2 = x.reshape(T, C)
    # x_pre[p, cc*T + t] = x[t, cc*128 + p]
    x_pre = np.ascontiguousarray(
        x2.reshape(T, NCC, 128).transpose(2, 1, 0).reshape(128, NCC * T)
    ).astype(BF)
    eye = np.eye(128, dtype=np.float32).astype(BF)
    ltri = np.ascontiguousarray(
        -1e5 * (np.arange(128)[None, :] > np.arange(128)[:, None])
    ).astype(BF)

    maps = []
    for core in range(NCORES):
        h0, h1 = HEAD_MAP[core]
        qcols = list(range(h0 * HS, (h0 + 1) * HS)) + \
                list(range(h1 * HS, (h1 + 1) * HS))
        # wqk_pre[p, cc*256 + j]: j<128 -> q cols, j>=128 -> k cols
        wq = W_attn[:, qcols]                          # [768, 128]
        wk = W_attn[:, [C + c for c in qcols]]         # [768, 128]
        wqk_cat = np.concatenate([wq, wk], axis=1)     # [768, 256]
        wqk_pre = np.ascontiguousarray(
            wqk_cat.reshape(NCC, 128, 256).transpose(1, 0, 2).reshape(128, -1)
        ).astype(BF)
        wvc = W_attn[:, [2 * C + c for c in qcols]]    # [768, 128]
        wv_pre = np.ascontiguousarray(
            wvc.reshape(NCC, 128, 128).transpose(1, 0, 2).reshape(128, -1)
        ).astype(BF)
        wpc = np.concatenate(
            [W_proj[h0 * HS:(h0 + 1) * HS, :],
             np.zeros_like(W_proj[:HS]) if h1 == h0
             else W_proj[h1 * HS:(h1 + 1) * HS, :]], axis=0)  # [128, 768]
        wp_pre = np.ascontiguousarray(wpc).astype(BF)
        bq = np.concatenate([b_attn[h0 * HS:(h0 + 1) * HS],
                             b_attn[h1 * HS:(h1 + 1) * HS]])
        bk = np.concatenate([b_attn[C + h0 * HS:C + (h0 + 1) * HS],
                             b_attn[C + h1 * HS:C + (h1 + 1) * HS]])
        bqk = np.ascontiguousarray(np.stack([bq, bk], axis=1).astype(np.float32))
        wqkx = np.concatenate([bqk.view(BF), wqk_pre], axis=1)
        cstx = np.concatenate([eye, ltri, wv_pre], axis=1)
        maps.append({
            "xd": x_pre, "wqk": np.ascontiguousarray(wqkx),
            "cst": np.ascontiguousarray(cstx), "wp": wp_pre,
        })
    return maps


def kernel(x, W_attn, b_attn, W_proj, b_proj, _trace=False, _trace_kwargs=None):
    x = np.asarray(x, np.float32)
    W_attn = np.asarray(W_attn, np.float32)
    b_attn = np.asarray(b_attn, np.float32)
    W_proj = np.asarray(W_proj, np.float32)
    b_proj = np.asarray(b_proj, np.float32)

    if "nc" not in _CACHE:
        _CACHE["nc"] = _build_program()
    nc = _CACHE["nc"]

    maps = _in_maps(x, W_attn, b_attn, W_proj, b_proj)
    kw = {}
    if _trace:
        kw = dict(trace=True, **(_trace_kwargs or {}))
    br = run_bass_kernel_spmd(nc, maps, list(range(NCORES)), **kw)
    acc = np.zeros((C, T), np.float64)
    for core in range(NCORES):
        o = np.asarray(br.results[core]["outp"]).astype(np.float64)
        acc += o.reshape(128, NCC, T).transpose(1, 0, 2).reshape(C, T)
    # exact host-side folds: b_proj, and the v-bias (softmax weights sum to 1)
    bv = b_attn[2 * C:]
    out_bias = b_proj.astype(np.float64) + bv.astype(np.float64) @ W_proj
    out = (acc.T + out_bias[None, :]).astype(np.float32)
    _CACHE["last_results"] = br
    return np.ascontiguousarray(out).reshape(1, T, C)


# revision 61
# speedup vs baseline: 1.3989x; 1.0149x over previous
"""TRN2 Bass kernel for nn_CausalSelfAttention_4054449128214.

The reference returns out_s + stop_gradient(out_full - out_s), whose forward
value is exactly out_full -- plain dense causal self-attention. So the kernel
computes: qkv = x@W_attn+b_attn, per-head causal softmax attention,
y@W_proj+b_proj.

Sharding (8 cores, no collectives):
  Megatron head-parallel. Cores 0-3 own head pairs (0,1)..(6,7); cores 4-7 own
  heads 8..11 (duplicated for SPMD shape-uniformity, second copy's W_proj rows
  zeroed). Each core computes its heads' QKV, attention, and a row-sliced
  partial output projection; the host sums the 8 partials (the Megatron
  row-parallel all-reduce).

v2 dataflow (all bf16 operands, fp32 PSUM accumulation):
  - Inputs host-converted to bf16, partition-major: every DMA is an identity
    copy with >=512B contiguous runs. x chunks stream in; QKV runs cc-major
    across 4 PSUM chains so compute starts with the first chunk.
  - K^T Q scores are causal-tight (valid query suffix per 128-key chunk),
    exp on the scalar engine writes bf16 e-tiles, diagonal masked by one
    bf16 tri multiply on DVE.
  - V^T comes straight from swapped-operand matmuls (no transposes).
  - PV uses the transposed orientation y^T[q,65] = e^T @ [V|1]: 65-row
    matmuls; softmax normalization is a per-partition reciprocal +
    tensor_scalar (no partition broadcast).
  - y^T transposes back per 128-query block on the PE; the projection
    contracts both heads in one 128-deep matmul, in four 256-query passes
    so stores overlap the attention tail.
  - q/k biases ride the mandatory PSUM->SBUF copies; the v bias and b_proj
    are folded on the host exactly (softmax weights sum to 1).
"""

import numpy as np
import ml_dtypes

import concourse.bacc as bacc
import concourse.mybir as mybir
import concourse.tile as tile
from concourse.bass_utils import run_bass_kernel_spmd

F32 = mybir.dt.float32
BF16 = mybir.dt.bfloat16
EXP = mybir.ActivationFunctionType.Exp
IDENT = mybir.ActivationFunctionType.Identity
COPY = mybir.ActivationFunctionType.Copy

T = 1024          # sequence length
C = 768           # channels
NH = 12           # heads
HS = 64           # head size
NCORES = 8
NCC = C // 128    # 6 contraction chunks
NKC = T // 128    # 8 key chunks of 128
SCALE = 1.0 / 8.0  # 1/sqrt(HS)
BF = np.dtype(ml_dtypes.bfloat16)

# core -> (head0, head1); cores 4-7 duplicate their head (2nd W_proj slice zeroed)
HEAD_MAP = [(0, 1), (2, 3), (4, 5), (6, 7), (8, 8), (9, 9), (10, 10), (11, 11)]

_CACHE: dict = {}


def _build_program():
    nc = bacc.Bacc("TRN2", target_bir_lowering=False, debug=False,
                   num_devices=NCORES)
    xd = nc.dram_tensor("xd", [128, NCC * T], BF16, kind="ExternalInput").ap()
    # packed row: bqk f32[2] | wqk bf16[1536]
    wqk = nc.dram_tensor("wqk", [128, 4 + NCC * 256], BF16,
                         kind="ExternalInput").ap()
    # packed row: eye bf16[128] | tri bf16[128] | wv bf16[768]
    cst = nc.dram_tensor("cst", [128, 1024], BF16, kind="ExternalInput").ap()
    wp = nc.dram_tensor("wp", [128, C], BF16, kind="ExternalInput").ap()
    outp = nc.dram_tensor("outp", [128, NCC * T], BF16, kind="ExternalOutput").ap()

    with tile.TileContext(nc) as tc:
        with (
            tc.tile_pool(name="const", bufs=1) as cp,
            tc.tile_pool(name="pm", bufs=2, space="PSUM") as pm,
            tc.tile_pool(name="psc", bufs=2, space="PSUM") as psc,
            tc.tile_pool(name="pvt", bufs=1, space="PSUM") as pvt,
            tc.tile_pool(name="pyp", bufs=1, space="PSUM") as pyp,
            tc.tile_pool(name="ptr", bufs=1, space="PSUM") as ptr,
        ):
            # vaug: per key-chunk [V_h0|1|V_h1|1] (65 cols per head), bf16
            vaug = cp.tile([128, NKC * 130], BF16, tag="vaug")
            va3 = vaug[:].rearrange("p (k j) -> p k j", k=2 * NKC)
            nc.gpsimd.memset(va3[:, :, 64:65], 1.0)

            # warmup: load the Exp table on Act and spin the PE p-state up
            # during the DMA window (dummy transposes on a memset scratch)
            wsc = cp.tile([128, 128], BF16, tag="wsc")
            nc.gpsimd.memset(wsc[:, 0:128], 0.0)
            wscf = cp.tile([128, 1], F32, tag="wscf")
            nc.vector.memset(wscf[:], 0.0)
            wsce = cp.tile([128, 1], BF16, tag="wsce")
            nc.scalar.activation(wsce[:], wscf[:], EXP, scale=1.0)

            # ---- big loads on the SP queue (identity copies) ----
            # DMA cadence floor is ~650ns (HWDGE setup), so pieces are >=2
            # chunks; tt0 halves of x stream first so the tt0 QKV chains,
            # scores, and exp start ~3us earlier.
            wqk_sb = cp.tile([128, 4 + NCC * 256], BF16, tag="wqk")
            bqk_sb = wqk_sb[:, 0:4].bitcast(F32)         # [128, 2] f32
            wqk3s = wqk_sb[:, 4:].rearrange("p (c j) -> p c j", c=NCC)
            x_sb = cp.tile([128, NCC * T], BF16, tag="x")
            x3 = x_sb[:].rearrange("p (c t) -> p c t", c=NCC)
            cst_sb = cp.tile([128, 1024], BF16, tag="cst")
            wp_sb = cp.tile([128, C], BF16, tag="wp")

            eye_sb = cst_sb[:, 0:128]                    # [128, 128] bf16
            ltri_sb = cst_sb[:, 128:256]                 # -1e5 upper tri
            wv3 = cst_sb[:, 256:1024].rearrange("p (c j) -> p c j", c=NCC)

            x4 = x_sb[:].rearrange("p (c t u) -> p c t u", c=NCC, t=2)
            xd4 = xd.rearrange("p (c t u) -> p c t u", c=NCC, t=2)
            nc.sync.dma_start(out=wqk_sb[:], in_=wqk)
            nc.sync.dma_start(out=x4[:, 0:2, 0], in_=xd4[:, 0:2, 0])
            nc.sync.dma_start(out=x4[:, 2:4, 0], in_=xd4[:, 2:4, 0])
            nc.sync.dma_start(out=x4[:, 4:6, 0], in_=xd4[:, 4:6, 0])
            nc.sync.dma_start(out=cst_sb[:], in_=cst)
            for c0 in (0, 2, 4):
                nc.sync.dma_start(out=x4[:, c0:c0 + 2, 1], in_=xd4[:, c0:c0 + 2, 1])
            nc.sync.dma_start(out=wp_sb[:], in_=wp)

            q_sb = cp.tile([128, T], BF16, tag="q_sb")
            k_sb = cp.tile([128, T], BF16, tag="k_sb")
            y_sb = cp.tile([128, T], BF16, tag="y_sb")
            # e tiles: per (head, kc), width = T - 128*kc
            e_t = [[cp.tile([128, T - 128 * kc], BF16, tag=f"e{h}_{kc}",
                            name=f"e{h}_{kc}") for kc in range(NKC)]
                   for h in range(2)]
            ytn = cp.tile([128, 256], BF16, tag="ytn")   # 4 regions of 64
            rec = cp.tile([128, 4], F32, tag="rec")      # 2 regions of 2
            ost = [cp.tile([128, NCC * 512], BF16, tag=f"ost{tt}", name=f"ost{tt}")
                   for tt in range(2)]
            # persistent PSUM tiles. HW zeroes a whole 2KB bank on matmul
            # start, so each bank holds exactly ONE live accumulation region,
            # reused in place (same-address WAR deps serialize reuse).
            vt_ps = pvt.tile([128, 128], F32, tag="vt")
            yp0_ps = pyp.tile([128, 65], F32, tag="yp0", name="yp0")
            yp1_ps = pyp.tile([128, 65], F32, tag="yp1", name="yp1")
            tr_ps = ptr.tile([128, 128], BF16, tag="tr")

            # PE p-state warmup: ~36 dummy transposes fill the DMA window so
            # QKV starts at full clock (ramp needs 3us of continuous busy)
            for _ in range(36):
                nc.tensor.transpose(tr_ps[:], wsc[:], wsc[:])

            # ---- QKV: cc-major chain pairs; pm bufs rotate tt0 -> tt1 ----
            def emit_qk(tt):
                ps = [pm.tile([128, 512], F32, tag="mm", name=f"q{tt}ps"),
                      pm.tile([128, 512], F32, tag="mm", name=f"k{tt}ps")]
                for cc in range(NCC):
                    for mt in range(2):
                        nc.tensor.matmul(
                            ps[mt][:], wqk3s[:, cc, mt * 128:(mt + 1) * 128],
                            x3[:, cc, tt * 512:(tt + 1) * 512],
                            start=(cc == 0), stop=(cc == NCC - 1))
                qdst = q_sb[:, tt * 512:(tt + 1) * 512]
                kdst = k_sb[:, tt * 512:(tt + 1) * 512]
                if tt == 0:
                    nc.scalar.activation(qdst, ps[0][:], IDENT,
                                         bias=bqk_sb[:, 0:1])
                    nc.vector.tensor_scalar_add(kdst[:, 0:128], ps[1][:, 0:128],
                                                bqk_sb[:, 1:2])
                    nc.vector.tensor_scalar_add(kdst[:, 128:512],
                                                ps[1][:, 128:512],
                                                bqk_sb[:, 1:2])
                else:
                    nc.vector.tensor_scalar_add(qdst, ps[0][:], bqk_sb[:, 0:1])
                    nc.vector.tensor_scalar_add(kdst, ps[1][:], bqk_sb[:, 1:2])

            def emit_vt(kc):
                # vT[key, (h0 hs | h1 hs)] for this 128-key chunk, then
                # scatter into vaug (the copy also frees the bank for kc+1)
                for cc in range(NCC):
                    nc.tensor.matmul(
                        vt_ps[:], x3[:, cc, kc * 128:(kc + 1) * 128], wv3[:, cc],
                        start=(cc == 0), stop=(cc == NCC - 1))
                dst = va3[:, 2 * kc:2 * kc + 2, 0:64]
                nc.vector.tensor_copy(dst,
                                      vt_ps[:].rearrange("p (h j) -> p h j", h=2))

            def emit_scores(h, kc, piece):
                # piece 0: q in [128kc, 512); piece 1: q in [512, 1024)
                # kc >= 4 only has piece 1 (q in [128kc, 1024))
                if kc < 4:
                    qlo, qhi = (128 * kc, 512) if piece == 0 else (512, T)
                else:
                    qlo, qhi = 128 * kc, T
                w = qhi - qlo
                diag = piece == 0 or kc >= 4
                ps = psc.tile([128, 512], F32, tag="sc")
                nc.tensor.matmul(
                    ps[:, 0:w], k_sb[h * 64:h * 64 + 64, kc * 128:kc * 128 + 128],
                    q_sb[h * 64:h * 64 + 64, qlo:qhi], start=True, stop=not diag)
                if diag:
                    # causal mask on the PE: accumulate -1e5 into the upper
                    # triangle of the diagonal block (ltri^T @ I)
                    nc.tensor.matmul(ps[:, 0:128], ltri_sb, eye_sb,
                                     start=False, stop=True)
                off = qlo - 128 * kc
                nc.scalar.activation(e_t[h][kc][:, off:off + w], ps[:, 0:w],
                                     EXP, scale=SCALE)

            y7 = [None, None]

            def yreg(h, qc):
                if qc == 7:
                    return y7[h][:, 0:65]
                if qc == 6:
                    return vt_ps[:, 0:65]
                return (yp0_ps if h == 0 else yp1_ps)[:]

            def emit_pv_mms(qc, kcs, last, heads=(0, 1)):
                for h in heads:
                    reg = yreg(h, qc)
                    for kc in kcs:
                        nc.tensor.matmul(
                            reg,
                            e_t[h][kc][:, 128 * (qc - kc):128 * (qc - kc) + 128],
                            vaug[:, 130 * kc + 65 * h:130 * kc + 65 * h + 65],
                            start=(kc == 0), stop=(last and kc == kcs[-1]))

            def emit_pv_norm1(qc, h):
                # reciprocal of the denominator column, then scale; late qcs
                # put h1 on Act (free after the exps) to shorten the DVE tail
                rr = (qc % 2) * 2
                nc.vector.reciprocal(rec[:, rr + h:rr + h + 1],
                                     yreg(h, qc)[:, 64:65])
                ydst = ytn[:, ((qc % 2) * 2 + h) * 64:((qc % 2) * 2 + h) * 64 + 64]
                if h == 1 and qc >= 6:
                    nc.scalar.activation(ydst, yreg(h, qc)[:, 0:64], COPY,
                                         scale=rec[:, rr + h:rr + h + 1])
                else:
                    nc.vector.tensor_scalar_mul(ydst, yreg(h, qc)[:, 0:64],
                                                rec[:, rr + h:rr + h + 1])

            def emit_pv_norm(qc):
                for h in range(2):
                    emit_pv_norm1(qc, h)

            def emit_pv(qc):
                # y^T[q, 0:64]=numerator, [:,64]=denominator, 128 queries/head
                if qc == 6:
                    # both heads share the vt bank: norm h0 before h1's chain
                    emit_pv_mms(qc, list(range(qc + 1)), True, heads=(0,))
                    emit_pv_norm1(qc, 0)
                    emit_pv_mms(qc, list(range(qc + 1)), True, heads=(1,))
                    emit_pv_norm1(qc, 1)
                else:
                    emit_pv_mms(qc, list(range(qc + 1)), True)
                    emit_pv_norm(qc)

            def emit_post(qc):
                # one transpose moves both heads' [q, hs] block to [hs2, q]
                nc.tensor.transpose(
                    tr_ps[:], ytn[:, (qc % 2) * 128:(qc % 2) * 128 + 128],
                    eye_sb)
                dst = y_sb[:, qc * 128:qc * 128 + 128]
                if qc >= 6:
                    nc.scalar.activation(dst, tr_ps[:], COPY)
                else:
                    nc.vector.tensor_copy(dst, tr_ps[:])

            def emit_proj(qp, engs=(nc.vector, nc.vector, nc.vector),
                          pool=None, qw=256, store="auto"):
                # one qw-query pass over all 6 row-chunks; 2 chunks share a
                # PSUM bank so each copy moves [128, 2*qw]
                qlo = qp * 256 if qw == 256 else qp * 128
                if store == "auto":
                    store = (qlo, qw)
                for eh in range(3):
                    p = pool[eh] if isinstance(pool, list) else (pool or pm)
                    ps = p.tile([128, 512], F32,
                                tag="mm" if p is pm else "sc")
                    for ei in range(2):
                        et = eh * 2 + ei
                        nc.tensor.matmul(
                            ps[:, ei * qw:(ei + 1) * qw],
                            wp_sb[:, et * 128:(et + 1) * 128],
                            y_sb[:, qlo:qlo + qw],
                            start=(ei == 0), stop=(ei == 1))
                    dst = ost[qlo // 512][:].rearrange(
                        "p (e t) -> p e t", e=NCC)[:, eh * 2:eh * 2 + 2,
                                                   qlo % 512:qlo % 512 + qw]
                    src = ps[:, 0:2 * qw].rearrange("p (e t) -> p e t", e=2)
                    eng = engs[eh]
                    if eng is nc.scalar:
                        nc.scalar.activation(dst, src, COPY)
                    else:
                        eng.tensor_copy(dst, src)
                if store is None:
                    return
                slo, sw = store
                outd = outp.rearrange("p (e t) -> p e t", e=NCC)
                osts = ost[slo // 512][:].rearrange("p (e t) -> p e t", e=NCC)
                nc.sync.dma_start(
                    out=outd[:, :, slo:slo + sw],
                    in_=osts[:, :, slo % 512:slo % 512 + sw])

            # ---------------- schedule ----------------
            emit_qk(0)
            for kc in range(3):
                for h in range(2):
                    emit_scores(h, kc, 0)
            emit_qk(1)
            for h in range(2):
                emit_scores(h, 3, 0)
            for h in range(2):
                emit_scores(h, 0, 1)
            emit_vt(0)
            emit_vt(1)
            emit_vt(2)
            emit_vt(3)
            emit_pv(0)
            emit_pv(1)
            emit_post(0)
            for h in range(2):
                emit_scores(h, 1, 1)
            emit_pv(2)
            emit_post(1)
            emit_proj(0)
            for h in range(2):
                emit_scores(h, 2, 1)
            for h in range(2):
                emit_scores(h, 3, 1)
            emit_pv(3)
            emit_post(2)
            for h in range(2):
                emit_scores(h, 4, 1)
            emit_vt(4)
            for h in range(2):
                emit_scores(h, 5, 1)
            emit_vt(5)
            for h in range(2):
                emit_scores(h, 6, 1)
            for h in range(2):
                emit_scores(h, 7, 1)
            emit_vt(6)
            emit_vt(7)
            emit_pv(4)
            emit_post(3)
            emit_proj(1)
            emit_pv(5)
            emit_post(4)
            emit_pv(6)
            emit_post(5)
            # chain 7 accumulates in freshly-freed psc banks so it does
            # not wait for the yp-bank norm ladder
            y7[0] = psc.tile([128, 512], F32, tag="sc", name="y7a")
            y7[1] = psc.tile([128, 512], F32, tag="sc", name="y7b")
            emit_pv(7)
            emit_post(6)
            emit_proj(2, engs=(nc.scalar, nc.vector, nc.vector), pool=psc)
            emit_proj(6, engs=(nc.scalar, nc.vector, nc.vector),
                      pool=[pm, pm, psc], qw=128, store=None)
            emit_post(7)
            emit_proj(7, engs=(nc.scalar, nc.vector, nc.scalar),
                      pool=[pm, pm, psc], qw=128, store=(768, 256))
    nc.compile()
    return nc


def _in_maps(x, W_attn, b_attn, W_proj, b_proj):
    x2 = x.reshape(T, C)
    # x_pre[p, cc*T + t] = x[t, cc*128 + p]
    x_pre = np.ascontiguousarray(
        x2.reshape(T, NCC, 128).transpose(2, 1, 0).reshape(128, NCC * T)
    ).astype(BF)
    eye = np.eye(128, dtype=np.float32).astype(BF)
    ltri = np.ascontiguousarray(
        -1e5 * (np.arange(128)[None, :] > np.arange(128)[:, None])
    ).astype(BF)

    maps = []
    for core in range(NCORES):
        h0, h1 = HEAD_MAP[core]
        qcols = list(range(h0 * HS, (h0 + 1) * HS)) + \
                list(range(h1 * HS, (h1 + 1) * HS))
        # wqk_pre[p, cc*256 + j]: j<128 -> q cols, j>=128 -> k cols
        wq = W_attn[:, qcols]                          # [768, 128]
        wk = W_attn[:, [C + c for c in qcols]]         # [768, 128]
        wqk_cat = np.concatenate([wq, wk], axis=1)     # [768, 256]
        wqk_pre = np.ascontiguousarray(
            wqk_cat.reshape(NCC, 128, 256).transpose(1, 0, 2).reshape(128, -1)
        ).astype(BF)
        wvc = W_attn[:, [2 * C + c for c in qcols]]    # [768, 128]
        wv_pre = np.ascontiguousarray(
            wvc.reshape(NCC, 128, 128).transpose(1, 0, 2).reshape(128, -1)
        ).astype(BF)
        wpc = np.concatenate(
            [W_proj[h0 * HS:(h0 + 1) * HS, :],
             np.zeros_like(W_proj[:HS]) if h1 == h0
             else W_proj[h1 * HS:(h1 + 1) * HS, :]], axis=0)  # [128, 768]
        wp_pre = np.ascontiguousarray(wpc).astype(BF)
        bq = np.concatenate([b_attn[h0 * HS:(h0 + 1) * HS],
                             b_attn[h1 * HS:(h1 + 1) * HS]])
        bk = np.concatenate([b_attn[C + h0 * HS:C + (h0 + 1) * HS],
                             b_attn[C + h1 * HS:C + (h1 + 1) * HS]])
        bqk = np.ascontiguousarray(np.stack([bq, bk], axis=1).astype(np.float32))
        wqkx = np.concatenate([bqk.view(BF), wqk_pre], axis=1)
        cstx = np.concatenate([eye, ltri, wv_pre], axis=1)
        maps.append({
            "xd": x_pre, "wqk": np.ascontiguousarray(wqkx),
            "cst": np.ascontiguousarray(cstx), "wp": wp_pre,
        })
    return maps


def kernel(x, W_attn, b_attn, W_proj, b_proj, _trace=False, _trace_kwargs=None):
    x = np.asarray(x, np.float32)
    W_attn = np.asarray(W_attn, np.float32)
    b_attn = np.asarray(b_attn, np.float32)
    W_proj = np.asarray(W_proj, np.float32)
    b_proj = np.asarray(b_proj, np.float32)

    if "nc" not in _CACHE:
        _CACHE["nc"] = _build_program()
    nc = _CACHE["nc"]

    maps = _in_maps(x, W_attn, b_attn, W_proj, b_proj)
    kw = {}
    if _trace:
        kw = dict(trace=True, **(_trace_kwargs or {}))
    br = run_bass_kernel_spmd(nc, maps, list(range(NCORES)), **kw)
    acc = np.zeros((C, T), np.float64)
    for core in range(NCORES):
        o = np.asarray(br.results[core]["outp"]).astype(np.float64)
        acc += o.reshape(128, NCC, T).transpose(1, 0, 2).reshape(C, T)
    # exact host-side folds: b_proj, and the v-bias (softmax weights sum to 1)
    bv = b_attn[2 * C:]
    out_bias = b_proj.astype(np.float64) + bv.astype(np.float64) @ W_proj
    out = (acc.T + out_bias[None, :]).astype(np.float32)
    _CACHE["last_results"] = br
    return np.ascontiguousarray(out).reshape(1, T, C)


# revision 64
# speedup vs baseline: 1.4023x; 1.0024x over previous
"""TRN2 Bass kernel for nn_CausalSelfAttention_4054449128214.

The reference returns out_s + stop_gradient(out_full - out_s), whose forward
value is exactly out_full -- plain dense causal self-attention. So the kernel
computes: qkv = x@W_attn+b_attn, per-head causal softmax attention,
y@W_proj+b_proj.

Sharding (8 cores, no collectives):
  Megatron head-parallel. Cores 0-3 own head pairs (0,1)..(6,7); cores 4-7 own
  heads 8..11 (duplicated for SPMD shape-uniformity, second copy's W_proj rows
  zeroed). Each core computes its heads' QKV, attention, and a row-sliced
  partial output projection; the host sums the 8 partials (the Megatron
  row-parallel all-reduce).

v2 dataflow (all bf16 operands, fp32 PSUM accumulation):
  - Inputs host-converted to bf16, partition-major: every DMA is an identity
    copy with >=512B contiguous runs. x chunks stream in; QKV runs cc-major
    across 4 PSUM chains so compute starts with the first chunk.
  - K^T Q scores are causal-tight (valid query suffix per 128-key chunk),
    exp on the scalar engine writes bf16 e-tiles, diagonal masked by one
    bf16 tri multiply on DVE.
  - V^T comes straight from swapped-operand matmuls (no transposes).
  - PV uses the transposed orientation y^T[q,65] = e^T @ [V|1]: 65-row
    matmuls; softmax normalization is a per-partition reciprocal +
    tensor_scalar (no partition broadcast).
  - y^T transposes back per 128-query block on the PE; the projection
    contracts both heads in one 128-deep matmul, in four 256-query passes
    so stores overlap the attention tail.
  - q/k biases ride the mandatory PSUM->SBUF copies; the v bias and b_proj
    are folded on the host exactly (softmax weights sum to 1).
"""

import numpy as np
import ml_dtypes

import concourse.bacc as bacc
import concourse.mybir as mybir
import concourse.tile as tile
from concourse.bass_utils import run_bass_kernel_spmd

F32 = mybir.dt.float32
BF16 = mybir.dt.bfloat16
EXP = mybir.ActivationFunctionType.Exp
IDENT = mybir.ActivationFunctionType.Identity
COPY = mybir.ActivationFunctionType.Copy

T = 1024          # sequence length
C = 768           # channels
NH = 12           # heads
HS = 64           # head size
NCORES = 8
NCC = C // 128    # 6 contraction chunks
NKC = T // 128    # 8 key chunks of 128
SCALE = 1.0 / 8.0  # 1/sqrt(HS)
BF = np.dtype(ml_dtypes.bfloat16)

# core -> (head0, head1); cores 4-7 duplicate their head (2nd W_proj slice zeroed)
HEAD_MAP = [(0, 1), (2, 3), (4, 5), (6, 7), (8, 8), (9, 9), (10, 10), (11, 11)]

_CACHE: dict = {}


def _build_program():
    nc = bacc.Bacc("TRN2", target_bir_lowering=False, debug=False,
                   num_devices=NCORES)
    xd = nc.dram_tensor("xd", [128, NCC * T], BF16, kind="ExternalInput").ap()
    # packed row: bqk f32[2] | wqk bf16[1536]
    wqk = nc.dram_tensor("wqk", [128, 4 + NCC * 256], BF16,
                         kind="ExternalInput").ap()
    # packed row: eye bf16[128] | tri bf16[128] | wv bf16[768]
    cst = nc.dram_tensor("cst", [128, 1024], BF16, kind="ExternalInput").ap()
    wp = nc.dram_tensor("wp", [128, C], BF16, kind="ExternalInput").ap()
    outp = nc.dram_tensor("outp", [128, NCC * T], BF16, kind="ExternalOutput").ap()

    with tile.TileContext(nc) as tc:
        with (
            tc.tile_pool(name="const", bufs=1) as cp,
            tc.tile_pool(name="pm", bufs=2, space="PSUM") as pm,
            tc.tile_pool(name="psc", bufs=2, space="PSUM") as psc,
            tc.tile_pool(name="pvt", bufs=1, space="PSUM") as pvt,
            tc.tile_pool(name="pyp", bufs=1, space="PSUM") as pyp,
            tc.tile_pool(name="ptr", bufs=1, space="PSUM") as ptr,
        ):
            # vaug: per key-chunk [V_h0|1|V_h1|1] (65 cols per head), bf16
            vaug = cp.tile([128, NKC * 130], BF16, tag="vaug")
            va3 = vaug[:].rearrange("p (k j) -> p k j", k=2 * NKC)
            nc.gpsimd.memset(va3[:, :, 64:65], 1.0)

            # warmup: load the Exp table on Act and spin the PE p-state up
            # during the DMA window (dummy transposes on a memset scratch)
            wsc = cp.tile([128, 128], BF16, tag="wsc")
            nc.gpsimd.memset(wsc[:, 0:128], 0.0)
            wscf = cp.tile([128, 1], F32, tag="wscf")
            nc.vector.memset(wscf[:], 0.0)
            wsce = cp.tile([128, 1], BF16, tag="wsce")
            nc.scalar.activation(wsce[:], wscf[:], EXP, scale=1.0)

            # ---- big loads on the SP queue (identity copies) ----
            # DMA cadence floor is ~650ns (HWDGE setup), so pieces are >=2
            # chunks; tt0 halves of x stream first so the tt0 QKV chains,
            # scores, and exp start ~3us earlier.
            wqk_sb = cp.tile([128, 4 + NCC * 256], BF16, tag="wqk")
            bqk_sb = wqk_sb[:, 0:4].bitcast(F32)         # [128, 2] f32
            wqk3s = wqk_sb[:, 4:].rearrange("p (c j) -> p c j", c=NCC)
            x_sb = cp.tile([128, NCC * T], BF16, tag="x")
            x3 = x_sb[:].rearrange("p (c t) -> p c t", c=NCC)
            cst_sb = cp.tile([128, 1024], BF16, tag="cst")
            wp_sb = cp.tile([128, C], BF16, tag="wp")

            eye_sb = cst_sb[:, 0:128]                    # [128, 128] bf16
            ltri_sb = cst_sb[:, 128:256]                 # -1e5 upper tri
            wv3 = cst_sb[:, 256:1024].rearrange("p (c j) -> p c j", c=NCC)

            x4 = x_sb[:].rearrange("p (c t u) -> p c t u", c=NCC, t=2)
            xd4 = xd.rearrange("p (c t u) -> p c t u", c=NCC, t=2)
            nc.sync.dma_start(out=wqk_sb[:], in_=wqk)
            nc.sync.dma_start(out=x4[:, 0:2, 0], in_=xd4[:, 0:2, 0])
            nc.sync.dma_start(out=x4[:, 2:4, 0], in_=xd4[:, 2:4, 0])
            nc.sync.dma_start(out=x4[:, 4:6, 0], in_=xd4[:, 4:6, 0])
            nc.sync.dma_start(out=cst_sb[:], in_=cst)
            for c0 in (0, 2, 4):
                nc.sync.dma_start(out=x4[:, c0:c0 + 2, 1], in_=xd4[:, c0:c0 + 2, 1])
            nc.sync.dma_start(out=wp_sb[:], in_=wp)

            q_sb = cp.tile([128, T], BF16, tag="q_sb")
            k_sb = cp.tile([128, T], BF16, tag="k_sb")
            y_sb = cp.tile([128, T], BF16, tag="y_sb")
            # e tiles: per (head, kc), width = T - 128*kc
            e_t = [[cp.tile([128, T - 128 * kc], BF16, tag=f"e{h}_{kc}",
                            name=f"e{h}_{kc}") for kc in range(NKC)]
                   for h in range(2)]
            ytn = cp.tile([128, 256], BF16, tag="ytn")   # 4 regions of 64
            rec = cp.tile([128, 4], F32, tag="rec")      # 2 regions of 2
            ost = [cp.tile([128, NCC * 512], BF16, tag=f"ost{tt}", name=f"ost{tt}")
                   for tt in range(2)]
            # persistent PSUM tiles. HW zeroes a whole 2KB bank on matmul
            # start, so each bank holds exactly ONE live accumulation region,
            # reused in place (same-address WAR deps serialize reuse).
            vt_ps = pvt.tile([128, 128], F32, tag="vt")
            yp0_ps = pyp.tile([128, 65], F32, tag="yp0", name="yp0")
            yp1_ps = pyp.tile([128, 65], F32, tag="yp1", name="yp1")
            tr_ps = ptr.tile([128, 128], BF16, tag="tr")

            # PE p-state warmup: ~36 dummy transposes fill the DMA window so
            # QKV starts at full clock (ramp needs 3us of continuous busy)
            for _ in range(36):
                nc.tensor.transpose(tr_ps[:], wsc[:], wsc[:])

            # ---- QKV: cc-major chain pairs; pm bufs rotate tt0 -> tt1 ----
            def emit_qk(tt):
                ps = [pm.tile([128, 512], F32, tag="mm", name=f"q{tt}ps"),
                      pm.tile([128, 512], F32, tag="mm", name=f"k{tt}ps")]
                for cc in range(NCC):
                    for mt in range(2):
                        nc.tensor.matmul(
                            ps[mt][:], wqk3s[:, cc, mt * 128:(mt + 1) * 128],
                            x3[:, cc, tt * 512:(tt + 1) * 512],
                            start=(cc == 0), stop=(cc == NCC - 1))
                qdst = q_sb[:, tt * 512:(tt + 1) * 512]
                kdst = k_sb[:, tt * 512:(tt + 1) * 512]
                if tt == 0:
                    nc.scalar.activation(qdst, ps[0][:], IDENT,
                                         bias=bqk_sb[:, 0:1])
                    nc.vector.tensor_scalar_add(kdst[:, 0:128], ps[1][:, 0:128],
                                                bqk_sb[:, 1:2])
                    nc.vector.tensor_scalar_add(kdst[:, 128:512],
                                                ps[1][:, 128:512],
                                                bqk_sb[:, 1:2])
                else:
                    nc.vector.tensor_scalar_add(qdst, ps[0][:], bqk_sb[:, 0:1])
                    nc.vector.tensor_scalar_add(kdst, ps[1][:], bqk_sb[:, 1:2])

            def emit_vt(kc):
                # vT[key, (h0 hs | h1 hs)] for this 128-key chunk, then
                # scatter into vaug (the copy also frees the bank for kc+1)
                for cc in range(NCC):
                    nc.tensor.matmul(
                        vt_ps[:], x3[:, cc, kc * 128:(kc + 1) * 128], wv3[:, cc],
                        start=(cc == 0), stop=(cc == NCC - 1))
                dst = va3[:, 2 * kc:2 * kc + 2, 0:64]
                nc.vector.tensor_copy(dst,
                                      vt_ps[:].rearrange("p (h j) -> p h j", h=2))

            def emit_scores(h, kc, piece):
                # piece 0: q in [128kc, 512); piece 1: q in [512, 1024)
                # kc >= 4 only has piece 1 (q in [128kc, 1024))
                if kc < 4:
                    qlo, qhi = (128 * kc, 512) if piece == 0 else (512, T)
                else:
                    qlo, qhi = 128 * kc, T
                w = qhi - qlo
                diag = piece == 0 or kc >= 4
                ps = psc.tile([128, 512], F32, tag="sc")
                nc.tensor.matmul(
                    ps[:, 0:w], k_sb[h * 64:h * 64 + 64, kc * 128:kc * 128 + 128],
                    q_sb[h * 64:h * 64 + 64, qlo:qhi], start=True, stop=not diag)
                if diag:
                    # causal mask on the PE: accumulate -1e5 into the upper
                    # triangle of the diagonal block (ltri^T @ I)
                    nc.tensor.matmul(ps[:, 0:128], ltri_sb, eye_sb,
                                     start=False, stop=True)
                off = qlo - 128 * kc
                nc.scalar.activation(e_t[h][kc][:, off:off + w], ps[:, 0:w],
                                     EXP, scale=SCALE)

            y7 = [None, None]

            def yreg(h, qc):
                if qc == 7:
                    return y7[h][:, 0:65]
                if qc == 6:
                    return vt_ps[:, 0:65]
                return (yp0_ps if h == 0 else yp1_ps)[:]

            def emit_pv_mms(qc, kcs, last, heads=(0, 1)):
                for h in heads:
                    reg = yreg(h, qc)
                    for kc in kcs:
                        nc.tensor.matmul(
                            reg,
                            e_t[h][kc][:, 128 * (qc - kc):128 * (qc - kc) + 128],
                            vaug[:, 130 * kc + 65 * h:130 * kc + 65 * h + 65],
                            start=(kc == 0), stop=(last and kc == kcs[-1]))

            def emit_pv_norm1(qc, h):
                # reciprocal of the denominator column, then scale; late qcs
                # put h1 on Act (free after the exps) to shorten the DVE tail
                rr = (qc % 2) * 2
                nc.vector.reciprocal(rec[:, rr + h:rr + h + 1],
                                     yreg(h, qc)[:, 64:65])
                ydst = ytn[:, ((qc % 2) * 2 + h) * 64:((qc % 2) * 2 + h) * 64 + 64]
                if qc >= 5 and (h == 1 or qc >= 6):
                    nc.scalar.activation(ydst, yreg(h, qc)[:, 0:64], COPY,
                                         scale=rec[:, rr + h:rr + h + 1])
                else:
                    nc.vector.tensor_scalar_mul(ydst, yreg(h, qc)[:, 0:64],
                                                rec[:, rr + h:rr + h + 1])

            def emit_pv_norm(qc):
                for h in range(2):
                    emit_pv_norm1(qc, h)

            def emit_pv(qc):
                # y^T[q, 0:64]=numerator, [:,64]=denominator, 128 queries/head
                if qc == 6:
                    # both heads share the vt bank: norm h0 before h1's chain
                    emit_pv_mms(qc, list(range(qc + 1)), True, heads=(0,))
                    emit_pv_norm1(qc, 0)
                    emit_pv_mms(qc, list(range(qc + 1)), True, heads=(1,))
                    emit_pv_norm1(qc, 1)
                else:
                    emit_pv_mms(qc, list(range(qc + 1)), True)
                    emit_pv_norm(qc)

            def emit_post(qc):
                # one transpose moves both heads' [q, hs] block to [hs2, q]
                nc.tensor.transpose(
                    tr_ps[:], ytn[:, (qc % 2) * 128:(qc % 2) * 128 + 128],
                    eye_sb)
                dst = y_sb[:, qc * 128:qc * 128 + 128]
                if qc >= 6:
                    nc.scalar.activation(dst, tr_ps[:], COPY)
                else:
                    nc.vector.tensor_copy(dst, tr_ps[:])

            def emit_proj(qp, engs=(nc.vector, nc.vector, nc.vector),
                          pool=None, qw=256, store="auto"):
                # one qw-query pass over all 6 row-chunks; 2 chunks share a
                # PSUM bank so each copy moves [128, 2*qw]
                qlo = qp * 256 if qw == 256 else qp * 128
                if store == "auto":
                    store = (qlo, qw)
                for eh in range(3):
                    p = pool[eh] if isinstance(pool, list) else (pool or pm)
                    ps = p.tile([128, 512], F32,
                                tag="mm" if p is pm else "sc")
                    for ei in range(2):
                        et = eh * 2 + ei
                        nc.tensor.matmul(
                            ps[:, ei * qw:(ei + 1) * qw],
                            wp_sb[:, et * 128:(et + 1) * 128],
                            y_sb[:, qlo:qlo + qw],
                            start=(ei == 0), stop=(ei == 1))
                    dst = ost[qlo // 512][:].rearrange(
                        "p (e t) -> p e t", e=NCC)[:, eh * 2:eh * 2 + 2,
                                                   qlo % 512:qlo % 512 + qw]
                    src = ps[:, 0:2 * qw].rearrange("p (e t) -> p e t", e=2)
                    eng = engs[eh]
                    if eng is nc.scalar:
                        nc.scalar.activation(dst, src, COPY)
                    else:
                        eng.tensor_copy(dst, src)
                if store is None:
                    return
                slo, sw = store
                outd = outp.rearrange("p (e t) -> p e t", e=NCC)
                osts = ost[slo // 512][:].rearrange("p (e t) -> p e t", e=NCC)
                nc.sync.dma_start(
                    out=outd[:, :, slo:slo + sw],
                    in_=osts[:, :, slo % 512:slo % 512 + sw])

            # ---------------- schedule ----------------
            emit_qk(0)
            for kc in range(3):
                for h in range(2):
                    emit_scores(h, kc, 0)
            emit_qk(1)
            for h in range(2):
                emit_scores(h, 3, 0)
            for h in range(2):
                emit_scores(h, 0, 1)
            emit_vt(0)
            emit_vt(1)
            emit_vt(2)
            emit_vt(3)
            emit_pv(0)
            emit_pv(1)
            emit_post(0)
            for h in range(2):
                emit_scores(h, 1, 1)
            emit_pv(2)
            emit_post(1)
            emit_proj(0)
            for h in range(2):
                emit_scores(h, 2, 1)
            for h in range(2):
                emit_scores(h, 3, 1)
            emit_pv(3)
            emit_post(2)
            for h in range(2):
                emit_scores(h, 4, 1)
            emit_vt(4)
            for h in range(2):
                emit_scores(h, 5, 1)
            emit_vt(5)
            for h in range(2):
                emit_scores(h, 6, 1)
            for h in range(2):
                emit_scores(h, 7, 1)
            emit_vt(6)
            emit_vt(7)
            emit_pv(4)
            emit_post(3)
            emit_proj(1)
            emit_pv(5)
            emit_post(4)
            emit_pv(6)
            emit_post(5)
            # chain 7 accumulates in freshly-freed psc banks so it does
            # not wait for the yp-bank norm ladder
            y7[0] = psc.tile([128, 512], F32, tag="sc", name="y7a")
            y7[1] = psc.tile([128, 512], F32, tag="sc", name="y7b")
            emit_pv(7)
            emit_post(6)
            emit_proj(2, engs=(nc.scalar, nc.vector, nc.vector), pool=psc)
            emit_proj(6, engs=(nc.scalar, nc.vector, nc.vector),
                      pool=[pm, pm, psc], qw=128, store=None)
            emit_post(7)
            emit_proj(7, engs=(nc.scalar, nc.vector, nc.scalar),
                      pool=[pm, pm, psc], qw=128, store=(768, 256))
    nc.compile()
    return nc


def _in_maps(x, W_attn, b_attn, W_proj, b_proj):
    x2 = x.reshape(T, C)
    # x_pre[p, cc*T + t] = x[t, cc*128 + p]
    x_pre = np.ascontiguousarray(
        x2.reshape(T, NCC, 128).transpose(2, 1, 0).reshape(128, NCC * T)
    ).astype(BF)
    eye = np.eye(128, dtype=np.float32).astype(BF)
    ltri = np.ascontiguousarray(
        -1e5 * (np.arange(128)[None, :] > np.arange(128)[:, None])
    ).astype(BF)

    maps = []
    for core in range(NCORES):
        h0, h1 = HEAD_MAP[core]
        qcols = list(range(h0 * HS, (h0 + 1) * HS)) + \
                list(range(h1 * HS, (h1 + 1) * HS))
        # wqk_pre[p, cc*256 + j]: j<128 -> q cols, j>=128 -> k cols
        wq = W_attn[:, qcols]                          # [768, 128]
        wk = W_attn[:, [C + c for c in qcols]]         # [768, 128]
        wqk_cat = np.concatenate([wq, wk], axis=1)     # [768, 256]
        wqk_pre = np.ascontiguousarray(
            wqk_cat.reshape(NCC, 128, 256).transpose(1, 0, 2).reshape(128, -1)
        ).astype(BF)
        wvc = W_attn[:, [2 * C + c for c in qcols]]    # [768, 128]
        wv_pre = np.ascontiguousarray(
            wvc.reshape(NCC, 128, 128).transpose(1, 0, 2).reshape(128, -1)
        ).astype(BF)
        wpc = np.concatenate(
            [W_proj[h0 * HS:(h0 + 1) * HS, :],
             np.zeros_like(W_proj[:HS]) if h1 == h0
             else W_proj[h1 * HS:(h1 + 1) * HS, :]], axis=0)  # [128, 768]
        wp_pre = np.ascontiguousarray(wpc).astype(BF)
        bq = np.concatenate([b_attn[h0 * HS:(h0 + 1) * HS],
                             b_attn[h1 * HS:(h1 + 1) * HS]])
        bk = np.concatenate([b_attn[C + h0 * HS:C + (h0 + 1) * HS],
                             b_attn[C + h1 * HS:C + (h1 + 1) * HS]])
        bqk = np.ascontiguousarray(np.stack([bq, bk], axis=1).astype(np.float32))
        wqkx = np.concatenate([bqk.view(BF), wqk_pre], axis=1)
        cstx = np.concatenate([eye, ltri, wv_pre], axis=1)
        maps.append({
            "xd": x_pre, "wqk": np.ascontiguousarray(wqkx),
            "cst": np.ascontiguousarray(cstx), "wp": wp_pre,
        })
    return maps


def kernel(x, W_attn, b_attn, W_proj, b_proj, _trace=False, _trace_kwargs=None):
    x = np.asarray(x, np.float32)
    W_attn = np.asarray(W_attn, np.float32)
    b_attn = np.asarray(b_attn, np.float32)
    W_proj = np.asarray(W_proj, np.float32)
    b_proj = np.asarray(b_proj, np.float32)

    if "nc" not in _CACHE:
        _CACHE["nc"] = _build_program()
    nc = _CACHE["nc"]

    maps = _in_maps(x, W_attn, b_attn, W_proj, b_proj)
    kw = {}
    if _trace:
        kw = dict(trace=True, **(_trace_kwargs or {}))
    br = run_bass_kernel_spmd(nc, maps, list(range(NCORES)), **kw)
    acc = np.zeros((C, T), np.float64)
    for core in range(NCORES):
        o = np.asarray(br.results[core]["outp"]).astype(np.float64)
        acc += o.reshape(128, NCC, T).transpose(1, 0, 2).reshape(C, T)
    # exact host-side folds: b_proj, and the v-bias (softmax weights sum to 1)
    bv = b_attn[2 * C:]
    out_bias = b_proj.astype(np.float64) + bv.astype(np.float64) @ W_proj
    out = (acc.T + out_bias[None, :]).astype(np.float32)
    _CACHE["last_results"] = br
    return np.ascontiguousarray(out).reshape(1, T, C)


# revision 71
# speedup vs baseline: 1.4028x; 1.0004x over previous
"""TRN2 Bass kernel for nn_CausalSelfAttention_4054449128214.

The reference returns out_s + stop_gradient(out_full - out_s), whose forward
value is exactly out_full -- plain dense causal self-attention. So the kernel
computes: qkv = x@W_attn+b_attn, per-head causal softmax attention,
y@W_proj+b_proj.

Sharding (8 cores, no collectives):
  Megatron head-parallel. Cores 0-3 own head pairs (0,1)..(6,7); cores 4-7 own
  heads 8..11 (duplicated for SPMD shape-uniformity, second copy's W_proj rows
  zeroed). Each core computes its heads' QKV, attention, and a row-sliced
  partial output projection; the host sums the 8 partials (the Megatron
  row-parallel all-reduce).

v2 dataflow (all bf16 operands, fp32 PSUM accumulation):
  - Inputs host-converted to bf16, partition-major: every DMA is an identity
    copy with >=512B contiguous runs. x chunks stream in; QKV runs cc-major
    across 4 PSUM chains so compute starts with the first chunk.
  - K^T Q scores are causal-tight (valid query suffix per 128-key chunk),
    exp on the scalar engine writes bf16 e-tiles, diagonal masked by one
    bf16 tri multiply on DVE.
  - V^T comes straight from swapped-operand matmuls (no transposes).
  - PV uses the transposed orientation y^T[q,65] = e^T @ [V|1]: 65-row
    matmuls; softmax normalization is a per-partition reciprocal +
    tensor_scalar (no partition broadcast).
  - y^T transposes back per 128-query block on the PE; the projection
    contracts both heads in one 128-deep matmul, in four 256-query passes
    so stores overlap the attention tail.
  - q/k biases ride the mandatory PSUM->SBUF copies; the v bias and b_proj
    are folded on the host exactly (softmax weights sum to 1).
"""

import numpy as np
import ml_dtypes

import concourse.bacc as bacc
import concourse.mybir as mybir
import concourse.tile as tile
from concourse.bass_utils import run_bass_kernel_spmd

F32 = mybir.dt.float32
BF16 = mybir.dt.bfloat16
EXP = mybir.ActivationFunctionType.Exp
IDENT = mybir.ActivationFunctionType.Identity
COPY = mybir.ActivationFunctionType.Copy

T = 1024          # sequence length
C = 768           # channels
NH = 12           # heads
HS = 64           # head size
NCORES = 8
NCC = C // 128    # 6 contraction chunks
NKC = T // 128    # 8 key chunks of 128
SCALE = 1.0 / 8.0  # 1/sqrt(HS)
BF = np.dtype(ml_dtypes.bfloat16)

# core -> (head0, head1); cores 4-7 duplicate their head (2nd W_proj slice zeroed)
HEAD_MAP = [(0, 1), (2, 3), (4, 5), (6, 7), (8, 8), (9, 9), (10, 10), (11, 11)]

_CACHE: dict = {}


def _build_program():
    nc = bacc.Bacc("TRN2", target_bir_lowering=False, debug=False,
                   num_devices=NCORES)
    xd = nc.dram_tensor("xd", [128, NCC * T], BF16, kind="ExternalInput").ap()
    # packed row: bqk f32[2] | wqk bf16[1536]
    wqk = nc.dram_tensor("wqk", [128, 4 + NCC * 256], BF16,
                         kind="ExternalInput").ap()
    # packed row: eye bf16[128] | tri bf16[128] | wv bf16[768]
    cst = nc.dram_tensor("cst", [128, 1024], BF16, kind="ExternalInput").ap()
    wp = nc.dram_tensor("wp", [128, C], BF16, kind="ExternalInput").ap()
    outp = nc.dram_tensor("outp", [128, NCC * T], BF16, kind="ExternalOutput").ap()

    with tile.TileContext(nc) as tc:
        with (
            tc.tile_pool(name="const", bufs=1) as cp,
            tc.tile_pool(name="pm", bufs=2, space="PSUM") as pm,
            tc.tile_pool(name="psc", bufs=2, space="PSUM") as psc,
            tc.tile_pool(name="pvt", bufs=1, space="PSUM") as pvt,
            tc.tile_pool(name="pyp", bufs=1, space="PSUM") as pyp,
            tc.tile_pool(name="ptr", bufs=1, space="PSUM") as ptr,
        ):
            # vaug: per key-chunk [V_h0|1|V_h1|1] (65 cols per head), bf16
            vaug = cp.tile([128, NKC * 130], BF16, tag="vaug")
            va3 = vaug[:].rearrange("p (k j) -> p k j", k=2 * NKC)
            nc.gpsimd.memset(va3[:, :, 64:65], 1.0)

            # warmup: load the Exp table on Act and spin the PE p-state up
            # during the DMA window (dummy transposes on a memset scratch)
            wsc = cp.tile([128, 128], BF16, tag="wsc")
            nc.gpsimd.memset(wsc[:, 0:128], 0.0)
            wscf = cp.tile([128, 1], F32, tag="wscf")
            nc.vector.memset(wscf[:], 0.0)
            wsce = cp.tile([128, 1], BF16, tag="wsce")
            nc.scalar.activation(wsce[:], wscf[:], EXP, scale=1.0)

            # ---- big loads on the SP queue (identity copies) ----
            # DMA cadence floor is ~650ns (HWDGE setup), so pieces are >=2
            # chunks; tt0 halves of x stream first so the tt0 QKV chains,
            # scores, and exp start ~3us earlier.
            wqk_sb = cp.tile([128, 4 + NCC * 256], BF16, tag="wqk")
            bqk_sb = wqk_sb[:, 0:4].bitcast(F32)         # [128, 2] f32
            wqk3s = wqk_sb[:, 4:].rearrange("p (c j) -> p c j", c=NCC)
            x_sb = cp.tile([128, NCC * T], BF16, tag="x")
            x3 = x_sb[:].rearrange("p (c t) -> p c t", c=NCC)
            cst_sb = cp.tile([128, 1024], BF16, tag="cst")
            wp_sb = cp.tile([128, C], BF16, tag="wp")

            eye_sb = cst_sb[:, 0:128]                    # [128, 128] bf16
            ltri_sb = cst_sb[:, 128:256]                 # -1e5 upper tri
            wv3 = cst_sb[:, 256:1024].rearrange("p (c j) -> p c j", c=NCC)

            x4 = x_sb[:].rearrange("p (c t u) -> p c t u", c=NCC, t=2)
            xd4 = xd.rearrange("p (c t u) -> p c t u", c=NCC, t=2)
            nc.sync.dma_start(out=wqk_sb[:], in_=wqk)
            nc.sync.dma_start(out=x4[:, 0:2, 0], in_=xd4[:, 0:2, 0])
            nc.sync.dma_start(out=x4[:, 2:4, 0], in_=xd4[:, 2:4, 0])
            nc.sync.dma_start(out=x4[:, 4:6, 0], in_=xd4[:, 4:6, 0])
            nc.sync.dma_start(out=cst_sb[:], in_=cst)
            for c0 in (0, 2, 4):
                nc.sync.dma_start(out=x4[:, c0:c0 + 2, 1], in_=xd4[:, c0:c0 + 2, 1])
            nc.sync.dma_start(out=wp_sb[:], in_=wp)

            q_sb = cp.tile([128, T], BF16, tag="q_sb")
            k_sb = cp.tile([128, T], BF16, tag="k_sb")
            y_sb = cp.tile([128, T], BF16, tag="y_sb")
            # e tiles: per (head, kc), width = T - 128*kc
            e_t = [[cp.tile([128, T - 128 * kc], BF16, tag=f"e{h}_{kc}",
                            name=f"e{h}_{kc}") for kc in range(NKC)]
                   for h in range(2)]
            ytn = cp.tile([128, 256], BF16, tag="ytn")   # 4 regions of 64
            rec = cp.tile([128, 4], F32, tag="rec")      # 2 regions of 2
            ost = [cp.tile([128, NCC * 512], BF16, tag=f"ost{tt}", name=f"ost{tt}")
                   for tt in range(2)]
            # persistent PSUM tiles. HW zeroes a whole 2KB bank on matmul
            # start, so each bank holds exactly ONE live accumulation region,
            # reused in place (same-address WAR deps serialize reuse).
            vt_ps = pvt.tile([128, 128], F32, tag="vt")
            yp0_ps = pyp.tile([128, 65], F32, tag="yp0", name="yp0")
            yp1_ps = pyp.tile([128, 65], F32, tag="yp1", name="yp1")
            tr_ps = ptr.tile([128, 128], BF16, tag="tr")

            # PE p-state warmup: ~36 dummy transposes fill the DMA window so
            # QKV starts at full clock (ramp needs 3us of continuous busy)
            for _ in range(36):
                nc.tensor.transpose(tr_ps[:], wsc[:], wsc[:])

            # ---- QKV: cc-major chain pairs; pm bufs rotate tt0 -> tt1 ----
            def emit_qk(tt):
                ps = [pm.tile([128, 512], F32, tag="mm", name=f"q{tt}ps"),
                      pm.tile([128, 512], F32, tag="mm", name=f"k{tt}ps")]
                for cc in range(NCC):
                    for mt in range(2):
                        nc.tensor.matmul(
                            ps[mt][:], wqk3s[:, cc, mt * 128:(mt + 1) * 128],
                            x3[:, cc, tt * 512:(tt + 1) * 512],
                            start=(cc == 0), stop=(cc == NCC - 1))
                qdst = q_sb[:, tt * 512:(tt + 1) * 512]
                kdst = k_sb[:, tt * 512:(tt + 1) * 512]
                if tt == 0:
                    nc.scalar.activation(qdst, ps[0][:], IDENT,
                                         bias=bqk_sb[:, 0:1])
                    nc.vector.tensor_scalar_add(kdst[:, 0:128], ps[1][:, 0:128],
                                                bqk_sb[:, 1:2])
                    nc.vector.tensor_scalar_add(kdst[:, 128:512],
                                                ps[1][:, 128:512],
                                                bqk_sb[:, 1:2])
                else:
                    nc.vector.tensor_scalar_add(qdst, ps[0][:], bqk_sb[:, 0:1])
                    nc.vector.tensor_scalar_add(kdst, ps[1][:], bqk_sb[:, 1:2])

            def emit_vt(kc):
                # vT[key, (h0 hs | h1 hs)] for this 128-key chunk, then
                # scatter into vaug (the copy also frees the bank for kc+1)
                for cc in range(NCC):
                    nc.tensor.matmul(
                        vt_ps[:], x3[:, cc, kc * 128:(kc + 1) * 128], wv3[:, cc],
                        start=(cc == 0), stop=(cc == NCC - 1))
                dst = va3[:, 2 * kc:2 * kc + 2, 0:64]
                nc.vector.tensor_copy(dst,
                                      vt_ps[:].rearrange("p (h j) -> p h j", h=2))

            def emit_scores(h, kc, piece):
                # piece 0: q in [128kc, 512); piece 1: q in [512, 1024)
                # kc >= 4 only has piece 1 (q in [128kc, 1024))
                if kc < 4:
                    qlo, qhi = (128 * kc, 512) if piece == 0 else (512, T)
                else:
                    qlo, qhi = 128 * kc, T
                w = qhi - qlo
                diag = piece == 0 or kc >= 4
                ps = psc.tile([128, 512], F32, tag="sc")
                nc.tensor.matmul(
                    ps[:, 0:w], k_sb[h * 64:h * 64 + 64, kc * 128:kc * 128 + 128],
                    q_sb[h * 64:h * 64 + 64, qlo:qhi], start=True, stop=not diag)
                if diag:
                    # causal mask on the PE: accumulate -1e5 into the upper
                    # triangle of the diagonal block (ltri^T @ I)
                    nc.tensor.matmul(ps[:, 0:128], ltri_sb, eye_sb,
                                     start=False, stop=True)
                off = qlo - 128 * kc
                nc.scalar.activation(e_t[h][kc][:, off:off + w], ps[:, 0:w],
                                     EXP, scale=SCALE)

            y7 = [None, None]

            def yreg(h, qc):
                if qc == 7:
                    return y7[h][:, 0:65]
                if qc == 6:
                    return vt_ps[:, 0:65]
                return (yp0_ps if h == 0 else yp1_ps)[:]

            def emit_pv_mms(qc, kcs, last, heads=(0, 1)):
                for h in heads:
                    reg = yreg(h, qc)
                    for kc in kcs:
                        nc.tensor.matmul(
                            reg,
                            e_t[h][kc][:, 128 * (qc - kc):128 * (qc - kc) + 128],
                            vaug[:, 130 * kc + 65 * h:130 * kc + 65 * h + 65],
                            start=(kc == 0), stop=(last and kc == kcs[-1]))

            def emit_pv_norm1(qc, h):
                # reciprocal of the denominator column, then scale; late qcs
                # put h1 on Act (free after the exps) to shorten the DVE tail
                rr = (qc % 2) * 2
                nc.vector.reciprocal(rec[:, rr + h:rr + h + 1],
                                     yreg(h, qc)[:, 64:65])
                ydst = ytn[:, ((qc % 2) * 2 + h) * 64:((qc % 2) * 2 + h) * 64 + 64]
                if qc >= 5 and (h == 1 or qc >= 6):
                    nc.scalar.activation(ydst, yreg(h, qc)[:, 0:64], COPY,
                                         scale=rec[:, rr + h:rr + h + 1])
                else:
                    nc.vector.tensor_scalar_mul(ydst, yreg(h, qc)[:, 0:64],
                                                rec[:, rr + h:rr + h + 1])

            def emit_pv_norm(qc):
                for h in range(2):
                    emit_pv_norm1(qc, h)

            def emit_pv(qc):
                # y^T[q, 0:64]=numerator, [:,64]=denominator, 128 queries/head
                if qc == 6:
                    # both heads share the vt bank: norm h0 before h1's chain
                    emit_pv_mms(qc, list(range(qc + 1)), True, heads=(0,))
                    emit_pv_norm1(qc, 0)
                    emit_pv_mms(qc, list(range(qc + 1)), True, heads=(1,))
                    emit_pv_norm1(qc, 1)
                else:
                    emit_pv_mms(qc, list(range(qc + 1)), True)
                    emit_pv_norm(qc)

            def emit_post(qc):
                # one transpose moves both heads' [q, hs] block to [hs2, q]
                nc.tensor.transpose(
                    tr_ps[:], ytn[:, (qc % 2) * 128:(qc % 2) * 128 + 128],
                    eye_sb)
                dst = y_sb[:, qc * 128:qc * 128 + 128]
                if qc >= 6:
                    nc.scalar.activation(dst, tr_ps[:], COPY)
                else:
                    nc.vector.tensor_copy(dst, tr_ps[:])

            def emit_proj(qp, engs=(nc.vector, nc.vector, nc.vector),
                          pool=None, qw=256, store="auto"):
                # one qw-query pass over all 6 row-chunks; 2 chunks share a
                # PSUM bank so each copy moves [128, 2*qw]
                qlo = qp * 256 if qw == 256 else qp * 128
                if store == "auto":
                    store = (qlo, qw)
                for eh in range(3):
                    p = pool[eh] if isinstance(pool, list) else (pool or pm)
                    ps = p.tile([128, 512], F32,
                                tag="mm" if p is pm else "sc")
                    for ei in range(2):
                        et = eh * 2 + ei
                        nc.tensor.matmul(
                            ps[:, ei * qw:(ei + 1) * qw],
                            wp_sb[:, et * 128:(et + 1) * 128],
                            y_sb[:, qlo:qlo + qw],
                            start=(ei == 0), stop=(ei == 1))
                    dst = ost[qlo // 512][:].rearrange(
                        "p (e t) -> p e t", e=NCC)[:, eh * 2:eh * 2 + 2,
                                                   qlo % 512:qlo % 512 + qw]
                    src = ps[:, 0:2 * qw].rearrange("p (e t) -> p e t", e=2)
                    eng = engs[eh]
                    if eng is nc.scalar:
                        nc.scalar.activation(dst, src, COPY)
                    else:
                        eng.tensor_copy(dst, src)
                if store is None:
                    return
                slo, sw = store
                outd = outp.rearrange("p (e t) -> p e t", e=NCC)
                osts = ost[slo // 512][:].rearrange("p (e t) -> p e t", e=NCC)
                nc.sync.dma_start(
                    out=outd[:, :, slo:slo + sw],
                    in_=osts[:, :, slo % 512:slo % 512 + sw])

            # ---------------- schedule ----------------
            emit_qk(0)
            for kc in range(3):
                for h in range(2):
                    emit_scores(h, kc, 0)
            emit_qk(1)
            for h in range(2):
                emit_scores(h, 3, 0)
            for h in range(2):
                emit_scores(h, 0, 1)
            emit_vt(0)
            emit_vt(1)
            emit_vt(2)
            emit_vt(3)
            emit_pv(0)
            emit_pv(1)
            emit_post(0)
            for h in range(2):
                emit_scores(h, 1, 1)
            emit_pv(2)
            emit_post(1)
            emit_proj(0)
            for h in range(2):
                emit_scores(h, 2, 1)
            for h in range(2):
                emit_scores(h, 3, 1)
            emit_pv(3)
            emit_post(2)
            for h in range(2):
                emit_scores(h, 4, 1)
            emit_vt(4)
            for h in range(2):
                emit_scores(h, 5, 1)
            emit_vt(5)
            for h in range(2):
                emit_scores(h, 6, 1)
            for h in range(2):
                emit_scores(h, 7, 1)
            emit_vt(6)
            emit_vt(7)
            emit_pv(4)
            emit_post(3)
            emit_proj(1)
            emit_pv(5)
            emit_post(4)
            emit_pv(6)
            emit_post(5)
            # chain 7 accumulates in freshly-freed psc banks so it does
            # not wait for the yp-bank norm ladder
            y7[0] = psc.tile([128, 512], F32, tag="sc", name="y7a")
            y7[1] = psc.tile([128, 512], F32, tag="sc", name="y7b")
            emit_pv(7)
            emit_post(6)
            emit_proj(2, engs=(nc.scalar, nc.vector, nc.vector), pool=psc)
            emit_proj(6, engs=(nc.scalar, nc.vector, nc.vector),
                      pool=[pm, pm, psc], qw=128, store=None)
            emit_post(7)
            emit_proj(7, engs=(nc.vector, nc.scalar, nc.vector),
                      pool=[pm, pm, psc], qw=128, store=(768, 256))
    nc.compile()
    return nc


def _in_maps(x, W_attn, b_attn, W_proj, b_proj):
    x2 = x.reshape(T, C)
    # x_pre[p, cc*T + t] = x[t, cc*128 + p]
    x_pre = np.ascontiguousarray(
        x2.reshape(T, NCC, 128).transpose(2, 1, 0).reshape(128, NCC * T)
    ).astype(BF)
    eye = np.eye(128, dtype=np.float32).astype(BF)
    ltri = np.ascontiguousarray(
        -1e5 * (np.arange(128)[None, :] > np.arange(128)[:, None])
    ).astype(BF)

    maps = []
    for core in range(NCORES):
        h0, h1 = HEAD_MAP[core]
        qcols = list(range(h0 * HS, (h0 + 1) * HS)) + \
                list(range(h1 * HS, (h1 + 1) * HS))
        # wqk_pre[p, cc*256 + j]: j<128 -> q cols, j>=128 -> k cols
        wq = W_attn[:, qcols]                          # [768, 128]
        wk = W_attn[:, [C + c for c in qcols]]         # [768, 128]
        wqk_cat = np.concatenate([wq, wk], axis=1)     # [768, 256]
        wqk_pre = np.ascontiguousarray(
            wqk_cat.reshape(NCC, 128, 256).transpose(1, 0, 2).reshape(128, -1)
        ).astype(BF)
        wvc = W_attn[:, [2 * C + c for c in qcols]]    # [768, 128]
        wv_pre = np.ascontiguousarray(
            wvc.reshape(NCC, 128, 128).transpose(1, 0, 2).reshape(128, -1)
        ).astype(BF)
        wpc = np.concatenate(
            [W_proj[h0 * HS:(h0 + 1) * HS, :],
             np.zeros_like(W_proj[:HS]) if h1 == h0
             else W_proj[h1 * HS:(h1 + 1) * HS, :]], axis=0)  # [128, 768]
        wp_pre = np.ascontiguousarray(wpc).astype(BF)
        bq = np.concatenate([b_attn[h0 * HS:(h0 + 1) * HS],
                             b_attn[h1 * HS:(h1 + 1) * HS]])
        bk = np.concatenate([b_attn[C + h0 * HS:C + (h0 + 1) * HS],
                             b_attn[C + h1 * HS:C + (h1 + 1) * HS]])
        bqk = np.ascontiguousarray(np.stack([bq, bk], axis=1).astype(np.float32))
        wqkx = np.concatenate([bqk.view(BF), wqk_pre], axis=1)
        cstx = np.concatenate([eye, ltri, wv_pre], axis=1)
        maps.append({
            "xd": x_pre, "wqk": np.ascontiguousarray(wqkx),
            "cst": np.ascontiguousarray(cstx), "wp": wp_pre,
        })
    return maps


def kernel(x, W_attn, b_attn, W_proj, b_proj, _trace=False, _trace_kwargs=None):
    x = np.asarray(x, np.float32)
    W_attn = np.asarray(W_attn, np.float32)
    b_attn = np.asarray(b_attn, np.float32)
    W_proj = np.asarray(W_proj, np.float32)
    b_proj = np.asarray(b_proj, np.float32)

    if "nc" not in _CACHE:
        _CACHE["nc"] = _build_program()
    nc = _CACHE["nc"]

    maps = _in_maps(x, W_attn, b_attn, W_proj, b_proj)
    kw = {}
    if _trace:
        kw = dict(trace=True, **(_trace_kwargs or {}))
    br = run_bass_kernel_spmd(nc, maps, list(range(NCORES)), **kw)
    acc = np.zeros((C, T), np.float64)
    for core in range(NCORES):
        o = np.asarray(br.results[core]["outp"]).astype(np.float64)
        acc += o.reshape(128, NCC, T).transpose(1, 0, 2).reshape(C, T)
    # exact host-side folds: b_proj, and the v-bias (softmax weights sum to 1)
    bv = b_attn[2 * C:]
    out_bias = b_proj.astype(np.float64) + bv.astype(np.float64) @ W_proj
    out = (acc.T + out_bias[None, :]).astype(np.float32)
    _CACHE["last_results"] = br
    return np.ascontiguousarray(out).reshape(1, T, C)
